# revision 1
# baseline (speedup 1.0000x reference)
"""Multi-head causal attention (B=2, S=2048, D=1024, H=16, Dh=64) on 8 TRN2 cores.

Sharding: tensor-parallel over 4 head-groups x data-parallel over 2 batches.
Core c handles batch c//4, heads [4*(c%4), 4*(c%4)+4). Each core computes its
partial output projection; the host sums the 4 partials per batch (the
"all-reduce") and adds b_O.

Per-core device program (Tile framework, fp32r matmuls, fp32 accumulation):
  QT/KT = (Wq|Wk/8)^T x^T   [dhead-pair=128, seq]   (scores scale folded into K)
  V     = x Wv              [seq, 4 heads x (64 | ones-col)]
  per (q-chunk 512, head):  S^T[kt] = KT_kt^T QT_chunk  (k on partitions)
      PT = exp(S^T) (ACT, kt-paired 1024-wide calls, causal-cropped,
                     triangular/extended mask on diagonal tiles)
      Z' = [V|1]^T PT accumulated over kt   -> rows 0:64 = Z^T, row 64 = denom
      ZT = Z' * (1/denom)   (DVE recip -> GPSIMD partition broadcast -> DVE mul)
  out[qtile, :] += ZT_pair^T Wo_pair        (partial over local heads)
"""

import numpy as np

import concourse.mybir as mybir
import concourse.tile as tile
from concourse import bacc
from concourse import bass_utils

F32 = mybir.dt.float32
F32R = mybir.dt.float32r

SEQ = 2048
DM = 1024
DH = 64
HLOC = 4          # heads per core
NKC = 8           # dmodel chunks of 128
NQC = 4           # q chunks of 512
QW = 512

_PROGRAMS = {}


def _build(with_bias: bool):
    nc = bacc.Bacc("TRN2", target_bir_lowering=False, debug=False, num_devices=8)

    xTa = nc.dram_tensor("xTa", [DM + 1, SEQ], F32R, kind="ExternalInput").ap()
    wq = nc.dram_tensor("wq", [DM, 256], F32R, kind="ExternalInput").ap()
    wk = nc.dram_tensor("wk", [DM, 256], F32R, kind="ExternalInput").ap()
    wv = nc.dram_tensor("wv", [DM, 256], F32R, kind="ExternalInput").ap()
    wo = nc.dram_tensor("wo", [256, DM], F32R, kind="ExternalInput").ap()
    # [128, 384]: cols 0:128 tri mask (q>=k), 128:256 zeros, 256:384 tri
    tri = nc.dram_tensor("tri", [128, 384], F32, kind="ExternalInput").ap()
    if with_bias:
        bqkv = nc.dram_tensor("bqkv", [1, 768], F32R, kind="ExternalInput").ap()
    out = nc.dram_tensor("out", [SEQ, DM], F32, kind="ExternalOutput").ap()

    with tile.TileContext(nc) as tc:
        with (
            tc.tile_pool(name="px", bufs=1) as px,
            tc.tile_pool(name="pw", bufs=1) as pw,
            tc.tile_pool(name="pqk", bufs=1) as pqk,
            tc.tile_pool(name="pv", bufs=1) as pv,
            tc.tile_pool(name="ppt", bufs=(4 if with_bias else 6)) as ppt,
            tc.tile_pool(name="pzt", bufs=4) as pzt,
            tc.tile_pool(name="prs", bufs=3) as prs,
            tc.tile_pool(name="pout", bufs=(3 if with_bias else 4)) as pout,
            tc.tile_pool(name="psS", bufs=3, space="PSUM") as psS,
            tc.tile_pool(name="psZ", bufs=2, space="PSUM") as psZ,
        ):
            # ---- load inputs (weights interleaved with x so the first
            # projection chain can start as soon as chunk 0 lands) ----
            WQ = pw.tile([128, NKC, 256], F32R, tag="wq")
            WK = pw.tile([128, NKC, 256], F32R, tag="wk")
            WV = pw.tile([128, NKC, 256], F32R, tag="wv")
            # x loaded in column blocks of 512, qc-block-major, so qc=0
            # projections (and attention) start after ~2MB of x instead of 8MB
            X = [[None] * NQC for _ in range(NKC)]
            for kc in range(NKC):
                nc.sync.dma_start(WQ[:, kc, :], wq[kc * 128:(kc + 1) * 128, :])
                nc.sync.dma_start(WK[:, kc, :], wk[kc * 128:(kc + 1) * 128, :])
                xt = px.tile([128, NQC, QW], F32R, tag=f"x{kc}", name=f"xt{kc}")
                for qb in range(NQC):
                    X[kc][qb] = xt[:, qb, :]
                nc.sync.dma_start(
                    X[kc][0], xTa[kc * 128:(kc + 1) * 128, 0:QW])
            for kc in range(NKC):
                nc.sync.dma_start(WV[:, kc, :], wv[kc * 128:(kc + 1) * 128, :])
            for qb in range(1, NQC):
                for kc in range(NKC):
                    nc.sync.dma_start(
                        X[kc][qb],
                        xTa[kc * 128:(kc + 1) * 128, qb * QW:(qb + 1) * QW])
            if with_bias:
                x_ones = px.tile([1, SEQ], F32R, tag="x8")
                nc.sync.dma_start(x_ones[:], xTa[DM:DM + 1, :])

            WO = pw.tile([128, 2, DM], F32R, tag="wo")
            for hp in range(2):
                nc.sync.dma_start(WO[:, hp, :], wo[hp * 128:(hp + 1) * 128, :])
            TRI = pw.tile([128, 384], F32, tag="tri")
            nc.sync.dma_start(TRI[:], tri)
            if with_bias:
                BQKV = pw.tile([1, 768], F32R, tag="bqkv")
                nc.sync.dma_start(BQKV[:], bqkv)

            ones4_f = pw.tile([128, HLOC, 1], F32, tag="ones4")
            nc.any.memset(ones4_f[:], 1.0)

            # ---- projections ----
            # QT/KT: [128 (head-pair), seq], per (hp, qc) tile of [128, 512].
            # Two chains share one 2-bank psum slot so more chains are in
            # flight while x is still streaming in.
            QT = [[None] * NQC for _ in range(2)]
            KT = [[None] * NQC for _ in range(2)]
            chains = []  # (w_sb, bias_off, dst, hp, qc), qc-major
            for qc in range(NQC):
                for w_sb, bias_off, dst in ((WQ, 0, QT), (WK, 256, KT)):
                    for hp in range(2):
                        chains.append((w_sb, bias_off, dst, hp, qc))
            # first two chains run as singles on the psZ banks, which are
            # otherwise idle until attention starts: 8 accumulation chains in
            # flight while x streams in instead of 6
            for j in (0, 1):
                w_sb, bias_off, dst, hp, qc = chains[j]
                pz = psZ.tile([128, QW], F32, tag="zo", name=f"qz{j}")
                for kc in range(NKC):
                    nc.tensor.matmul(
                        pz[:],
                        w_sb[:, kc, hp * 128:(hp + 1) * 128],
                        X[kc][qc],
                        start=(kc == 0),
                        stop=(kc == NKC - 1 and not with_bias),
                    )
                if with_bias:
                    nc.tensor.matmul(
                        pz[:],
                        BQKV[0:1, bias_off + hp * 128:bias_off + (hp + 1) * 128],
                        x_ones[0:1, qc * QW:(qc + 1) * QW],
                        start=False, stop=True,
                    )
                t = pqk.tile([128, QW], F32R,
                             tag=f"{'q' if dst is QT else 'k'}{hp}_{qc}",
                             name=f"tz_{bias_off}_{hp}_{qc}")
                nc.scalar.copy(t[:], pz[:])
                dst[hp][qc] = t
            V1 = [None] * 16

            def emit_qk_pair(j):
                pp = psS.tile([128, 2, QW], F32, tag="s", name=f"qk{j}")
                for kc in range(NKC):
                    for i in (0, 1):
                        w_sb, bias_off, dst, hp, qc = chains[j + i]
                        nc.tensor.matmul(
                            pp[:, i, :],
                            w_sb[:, kc, hp * 128:(hp + 1) * 128],
                            X[kc][qc],
                            start=(kc == 0),
                            stop=(kc == NKC - 1 and not with_bias),
                        )
                for i in (0, 1):
                    w_sb, bias_off, dst, hp, qc = chains[j + i]
                    if with_bias:
                        nc.tensor.matmul(
                            pp[:, i, :],
                            BQKV[0:1, bias_off + hp * 128:
                                 bias_off + (hp + 1) * 128],
                            x_ones[0:1, qc * QW:(qc + 1) * QW],
                            start=False, stop=True,
                        )
                    t = pqk.tile([128, QW], F32R,
                                 tag=f"{'q' if dst is QT else 'k'}{hp}_{qc}",
                                 name=f"t_{bias_off}_{hp}_{qc}")
                    nc.scalar.copy(t[:], pp[:, i, :])
                    dst[hp][qc] = t

            def emit_v_pair(st2):
                # V: [128, 4 heads, 65] per seq-tile (col 64 = ones). Each
                # chain padded to a full psum bank (two accumulation groups
                # must not share a bank).
                pp = psS.tile([128, 2, QW], F32, tag="s", name=f"vq{st2}")
                for kc in range(NKC):
                    for i in (0, 1):
                        st = st2 * 2 + i
                        nc.tensor.matmul(
                            pp[:, i, 0:256],
                            X[kc][st // 4][:, (st % 4) * 128:
                                           (st % 4 + 1) * 128],
                            WV[:, kc, :],
                            start=(kc == 0),
                            stop=(kc == NKC - 1 and not with_bias),
                        )
                for i in (0, 1):
                    st = st2 * 2 + i
                    if with_bias:
                        nc.tensor.matmul(
                            pp[:, i, 0:256],
                            x_ones[0:1, st * 128:(st + 1) * 128],
                            BQKV[0:1, 512:768],
                            start=False, stop=True,
                        )
                    vt = pv.tile([128, HLOC, DH + 1], F32R, tag=f"v{st}",
                                 name=f"vt{st}")
                    nc.vector.tensor_copy(
                        vt[:, :, 0:DH],
                        pp[:, i, 0:256].rearrange("p (h d) -> p h d", h=HLOC),
                    )
                    nc.vector.tensor_copy(vt[:, :, DH:DH + 1], ones4_f[:])
                    V1[st] = vt

            # interleave: per qc-block, its QK pairs then its V pairs, so the
            # qc=0 inputs of attention complete first
            for qc in range(NQC):
                j0 = 4 * qc
                for j in range(max(2, j0), j0 + 4, 2):
                    emit_qk_pair(j)
                emit_v_pair(2 * qc)
                emit_v_pair(2 * qc + 1)

            # ---- attention + output projection, per q-chunk ----
            for qc in range(NQC):
                q0 = qc * QW
                nkt = 4 * qc + 4
                ZT = [None, None]  # per head-pair [128, 512]
                OSB = [None] * 4
                for h in range(HLOC):
                    hp, hh = h // 2, h % 2
                    zps = psZ.tile([128, QW], F32, tag="zo",
                                   name=f"z{qc}_{h}")
                    for ktp in range(nkt // 2):
                        if True:
                            sps = psS.tile([128, 2, QW], F32, tag="s",
                                           name=f"s{qc}_{h}_{ktp}")
                            offs = []
                            for i in (0, 1):
                                kt = 2 * ktp + i
                                # crop to causal region; keep matmul N >= 256
                                # (fp32r below 256 runs at 1/4 rate)
                                off = min(max(0, kt * 128 - q0), 256)
                                offs.append(off)
                                nc.tensor.matmul(
                                    sps[:, i, off:QW],
                                    KT[hp][kt // 4][hh * DH:(hh + 1) * DH,
                                                    (kt % 4) * 128:(kt % 4 + 1) * 128],
                                    QT[hp][qc][hh * DH:(hh + 1) * DH, off:QW],
                                    start=True, stop=True,
                                )
                            pt = ppt.tile([128, 2, QW], F32R, tag="pt",
                                          name=f"pt{qc}_{h}_{ktp}")
                            if offs[0] == 0 and offs[1] == 0:
                                nc.scalar.activation(
                                    pt[:], sps[:],
                                    mybir.ActivationFunctionType.Exp,
                                )
                            else:
                                for i in (0, 1):
                                    nc.scalar.activation(
                                        pt[:, i, offs[i]:QW], sps[:, i, offs[i]:QW],
                                        mybir.ActivationFunctionType.Exp,
                                    )
                        for i in (0, 1):
                            kt = 2 * ktp + i
                            off = offs[i]
                            if kt >= nkt - 4:  # diagonal: mask
                                moff = kt * 128 - q0  # true mask offset
                                if moff == 384:
                                    # cols 256:384 masked to 0, tri on 384:512
                                    nc.vector.tensor_mul(
                                        pt[:, i, 256:512],
                                        pt[:, i, 256:512], TRI[:, 128:384]
                                    )
                                else:
                                    nc.vector.tensor_mul(
                                        pt[:, i, moff:moff + 128],
                                        pt[:, i, moff:moff + 128], TRI[:, 0:128]
                                    )
                            nc.tensor.matmul(
                                zps[0:DH + 1, off:QW],
                                V1[kt][:, h, :],
                                pt[:, i, off:QW],
                                start=(kt == 0),
                                stop=(kt == nkt - 1),
                                skip_group_check=True,
                            )
                    # normalize: ZT[0:64] = zps[0:64] / zps[64]
                    recip = prs.tile([1, QW], F32R, tag="recip",
                                     name=f"rc{qc}_{h}")
                    with nc.allow_low_precision(reason="softmax recip in fp32r"):
                        nc.vector.reciprocal(recip[:], zps[DH:DH + 1, :])
                    rb = prs.tile([DH, QW], F32R, tag="rb", name=f"rb{qc}_{h}")
                    nc.gpsimd.partition_broadcast(rb[:], recip[:])
                    if ZT[hp] is None:
                        ZT[hp] = pzt.tile([128, QW], F32R, tag="zt",
                                          name=f"zt{qc}_{hp}")
                    nc.vector.tensor_mul(
                        ZT[hp][hh * DH:(hh + 1) * DH, :], zps[0:DH, :], rb[:]
                    )

                # out[q0:q0+512, :] = sum_hp ZT[hp].T @ WO[hp]
                for qt in range(4):
                    osb = pout.tile([128, DM], F32, tag="ob",
                                    name=f"ob{qc}_{qt}")
                    for mc in range(2):
                        ops = psZ.tile([128, QW], F32, tag="zo",
                                       name=f"o{qc}_{qt}_{mc}")
                        for hp in range(2):
                            nc.tensor.matmul(
                                ops[:],
                                ZT[hp][:, qt * 128:(qt + 1) * 128],
                                WO[:, hp, mc * QW:(mc + 1) * QW],
                                start=(hp == 0), stop=(hp == 1),
                            )
                        nc.vector.tensor_copy(osb[:, mc * QW:(mc + 1) * QW], ops[:])
                        nc.sync.dma_start(
                            out[q0 + qt * 128:q0 + (qt + 1) * 128,
                                mc * QW:(mc + 1) * QW],
                            osb[:, mc * QW:(mc + 1) * QW],
                        )


    nc.compile()
    return nc


def _get_program(with_bias: bool):
    if with_bias not in _PROGRAMS:
        _PROGRAMS[with_bias] = _build(with_bias)
    return _PROGRAMS[with_bias]


def kernel(normalized_resid_pre, W_Q, W_K, W_V, W_O, b_Q, b_K, b_V, b_O):
    x = np.asarray(normalized_resid_pre, dtype=np.float32)
    W_Q = np.asarray(W_Q, dtype=np.float32)
    W_K = np.asarray(W_K, dtype=np.float32)
    W_V = np.asarray(W_V, dtype=np.float32)
    W_O = np.asarray(W_O, dtype=np.float32)
    b_Q = np.asarray(b_Q, dtype=np.float32)
    b_K = np.asarray(b_K, dtype=np.float32)
    b_V = np.asarray(b_V, dtype=np.float32)
    b_O = np.asarray(b_O, dtype=np.float32)

    batch, seq, dm = x.shape
    with_bias = bool(np.any(b_Q) or np.any(b_K) or np.any(b_V))
    nc = _get_program(with_bias)

    tri1 = np.triu(np.ones((128, 128), dtype=np.float32))
    tri = np.ascontiguousarray(np.concatenate(
        [tri1, np.zeros((128, 128), dtype=np.float32), tri1], axis=1
    ))
    in_maps = []
    for c in range(8):
        b, g = c // 4, c % 4
        hs = slice(4 * g, 4 * g + 4)
        xT = x[b].T  # [1024, 2048]
        xTa = np.concatenate(
            [xT, np.ones((1, seq), dtype=np.float32)], axis=0
        )
        m = {
            "xTa": np.ascontiguousarray(xTa),
            "wq": np.ascontiguousarray(
                np.transpose(W_Q[hs], (1, 0, 2)).reshape(dm, 256)),
            "wk": np.ascontiguousarray(
                np.transpose(W_K[hs], (1, 0, 2)).reshape(dm, 256) * 0.125),
            "wv": np.ascontiguousarray(
                np.transpose(W_V[hs], (1, 0, 2)).reshape(dm, 256)),
            "wo": np.ascontiguousarray(W_O[hs].reshape(256, dm)),
            "tri": tri,
        }
        if with_bias:
            m["bqkv"] = np.ascontiguousarray(np.concatenate(
                [b_Q[hs].reshape(256), b_K[hs].reshape(256) * 0.125,
                 b_V[hs].reshape(256)]
            )[None, :].astype(np.float32))
        in_maps.append(m)

    res = bass_utils.run_bass_kernel_spmd(nc, in_maps, core_ids=list(range(8)))
    parts = [res.results[c]["out"] for c in range(8)]
    full = np.stack(
        [parts[0] + parts[1] + parts[2] + parts[3],
         parts[4] + parts[5] + parts[6] + parts[7]]
    )
    full += b_O
    return full.astype(np.float32)



# revision 3
# speedup vs baseline: 1.0096x; 1.0096x over previous
"""Multi-head causal attention (B=2, S=2048, D=1024, H=16, Dh=64) on 8 TRN2 cores.

Sharding: tensor-parallel over 4 head-groups x data-parallel over 2 batches.
Core c handles batch c//4, heads [4*(c%4), 4*(c%4)+4). Each core computes its
partial output projection; the host sums the 4 partials per batch (the
"all-reduce") and adds b_O.

Mixed-precision device program (fp32 PSUM accumulation everywhere):
  - Q/K/V projections: fp8e4m3 DoubleRow matmuls (0.5 cyc/row, 256-deep
    contraction per instr). Host pre-splits x and 64*W into hi/lo fp8 pairs;
    3 compensation terms xh@Wh + xl@Wh + xh@Wl recover ~bf16 accuracy.
  - Scores S = (64q)(64k): bf16 QT/KT, per-128-column causal crops. The /8
    softmax scale and the 1/4096 fp8 weight scaling fold into the ACT exp
    (scale=1/32768).
  - P = exp(S): ACT writes fp8 tiles for off-diagonal k-pairs, bf16 for the
    4 diagonal k-tiles (where P concentrates and fp8 noise would not average
    out). Triangular masks multiply bf16 diag tiles only (DVE 2x on bf16).
  - Z = P @ [64V|1]: off-diag via fp8 DoubleRow over k-tile pairs (0.25
    cyc/row/k-tile), diag via bf16-moving matmuls; row 64 accumulates the
    softmax denominator.
  - out += (Z/denom)^T @ (Wo/64): f32r stationary ZT, bf16 moving WO.
"""

import numpy as np
import ml_dtypes

import concourse.mybir as mybir
import concourse.tile as tile
from concourse import bacc
from concourse import bass_utils

F32 = mybir.dt.float32
F32R = mybir.dt.float32r
BF16 = mybir.dt.bfloat16
F8 = mybir.dt.float8e4

SEQ = 2048
DM = 1024
DH = 64
HLOC = 4          # heads per core
KCP = 4           # dmodel pair-chunks of 256
NQC = 4           # q chunks of 512
QW = 512
WS = 64.0         # fp8 weight scale
EXP_SCALE = 1.0 / (WS * WS * 8.0)
DR = mybir.MatmulPerfMode.DoubleRow

_PROGRAMS = {}


def _build(with_bias: bool):
    nc = bacc.Bacc("TRN2", target_bir_lowering=False, debug=False, num_devices=8)

    # [128, j(4), i(2), qb(4), 512] with element [p,j,i,qb,s] = xT[256j+128i+p,
    # 512qb+s]; hi/lo fp8 split of xT
    xh8 = nc.dram_tensor("xh8", [128, KCP, 2, NQC, QW], F8, kind="ExternalInput").ap()
    xl8 = nc.dram_tensor("xl8", [128, KCP, 2, NQC, QW], F8, kind="ExternalInput").ap()
    # [128, j(4), i(2), 256]: hi/lo of 64*W[256j+128i+p, m]
    wname = lambda n: nc.dram_tensor(n, [128, KCP, 2, 256], F8, kind="ExternalInput").ap()
    wqh, wql = wname("wqh"), wname("wql")
    wkh, wkl = wname("wkh"), wname("wkl")
    wvh, wvl = wname("wvh"), wname("wvl")
    wo = nc.dram_tensor("wo", [256, DM], BF16, kind="ExternalInput").ap()
    tri = nc.dram_tensor("tri", [128, 128], BF16, kind="ExternalInput").ap()
    if with_bias:
        bqkv = nc.dram_tensor("bqkv", [1, 768], BF16, kind="ExternalInput").ap()
        ones = nc.dram_tensor("ones", [1, SEQ], BF16, kind="ExternalInput").ap()
    out = nc.dram_tensor("out", [SEQ, DM], F32, kind="ExternalOutput").ap()

    with tile.TileContext(nc) as tc:
        with (
            tc.tile_pool(name="px", bufs=1) as px,
            tc.tile_pool(name="pw", bufs=1) as pw,
            tc.tile_pool(name="pqk", bufs=1) as pqk,
            tc.tile_pool(name="pv", bufs=1) as pv,
            tc.tile_pool(name="ppt8", bufs=4) as ppt8,
            tc.tile_pool(name="pptb", bufs=4) as pptb,
            tc.tile_pool(name="pzt", bufs=4) as pzt,
            tc.tile_pool(name="prs", bufs=3) as prs,
            tc.tile_pool(name="pout", bufs=4) as pout,
            tc.tile_pool(name="psS", bufs=2, space="PSUM") as psS,
            tc.tile_pool(name="psZ", bufs=2, space="PSUM") as psZ,
            tc.tile_pool(name="psO", bufs=2, space="PSUM") as psO,
        ):
            # ---- SBUF tiles ----
            XH = px.tile([128, KCP, 2, NQC, QW], F8, tag="xh")
            XL = px.tile([128, KCP, 2, NQC, QW], F8, tag="xl")
            WQh = pw.tile([128, KCP, 2, 256], F8, tag="wqh")
            WQl = pw.tile([128, KCP, 2, 256], F8, tag="wql")
            WKh = pw.tile([128, KCP, 2, 256], F8, tag="wkh")
            WKl = pw.tile([128, KCP, 2, 256], F8, tag="wkl")
            WVh = pw.tile([128, KCP, 2, 256], F8, tag="wvh")
            WVl = pw.tile([128, KCP, 2, 256], F8, tag="wvl")

            # ---- input DMA: qb0 x-blocks + QK weights first ----
            for w_sb, w_dr in ((WQh, wqh), (WQl, wql), (WKh, wkh), (WKl, wkl)):
                nc.sync.dma_start(w_sb[:], w_dr)
            for j in range(KCP):
                nc.sync.dma_start(XH[:, j, :, 0, :], xh8[:, j, :, 0, :])
                nc.sync.dma_start(XL[:, j, :, 0, :], xl8[:, j, :, 0, :])
            nc.sync.dma_start(WVh[:], wvh)
            nc.sync.dma_start(WVl[:], wvl)
            TRI = pw.tile([128, 128], BF16, tag="tri")
            nc.sync.dma_start(TRI[:], tri)
            for qb in range(1, NQC):
                for j in range(KCP):
                    nc.sync.dma_start(XH[:, j, :, qb, :], xh8[:, j, :, qb, :])
                    nc.sync.dma_start(XL[:, j, :, qb, :], xl8[:, j, :, qb, :])
            WO = pw.tile([128, 2, DM], BF16, tag="wo")
            for hp in range(2):
                nc.sync.dma_start(WO[:, hp, :], wo[hp * 128:(hp + 1) * 128, :])
            if with_bias:
                BQKV = pw.tile([1, 768], BF16, tag="bqkv")
                nc.sync.dma_start(BQKV[:], bqkv)
                ONES = pw.tile([1, SEQ], BF16, tag="ones")
                nc.sync.dma_start(ONES[:], ones)

            # QT/KT bf16 [128 (head-pair), qc, 512] per hp; VB f32r with ones
            # col; V8 fp8 pairs (st=2m+i, only m<6 used off-diagonal)
            QT = [pqk.tile([128, NQC, QW], BF16, tag=f"qt{hp}", name=f"qt{hp}")
                  for hp in range(2)]
            KT = [pqk.tile([128, NQC, QW], BF16, tag=f"kt{hp}", name=f"kt{hp}")
                  for hp in range(2)]
            VB = pv.tile([128, 16, HLOC, DH + 1], F32R, tag="vb")
            V8 = pv.tile([128, 6, 2, HLOC, DH + 1], F8, tag="v8")
            nc.vector.memset(VB[:, :, :, DH:DH + 1], 1.0)
            nc.vector.memset(V8[:, :, :, :, DH:DH + 1], 1.0)

            # ---- projections: fp8 DoubleRow 3-term chains ----
            def qk_chain(pp_slice, w_hi, w_lo, hp, qc, bias_off):
                for j in range(KCP):
                    for term, (w_sb, x_sb) in enumerate(
                        ((w_hi, XH), (w_hi, XL), (w_lo, XH))
                    ):
                        nc.tensor.matmul(
                            pp_slice,
                            w_sb[:, j, :, hp * 128:(hp + 1) * 128],
                            x_sb[:, j, :, qc, :],
                            start=(j == 0 and term == 0),
                            stop=(j == KCP - 1 and term == 2 and not with_bias),
                            perf_mode=DR,
                        )
                if with_bias:
                    nc.tensor.matmul(
                        pp_slice,
                        BQKV[0:1, bias_off + hp * 128:bias_off + (hp + 1) * 128],
                        ONES[0:1, qc * QW:(qc + 1) * QW],
                        start=False, stop=True,
                    )

            def v_chain(pp_slice, st):
                qb, s0 = st // 4, (st % 4) * 128
                for j in range(KCP):
                    for term, (x_sb, w_sb) in enumerate(
                        ((XH, WVh), (XL, WVh), (XH, WVl))
                    ):
                        nc.tensor.matmul(
                            pp_slice,
                            x_sb[:, j, :, qb, s0:s0 + 128],
                            w_sb[:, j, :, :],
                            start=(j == 0 and term == 0),
                            stop=(j == KCP - 1 and term == 2 and not with_bias),
                            perf_mode=DR,
                        )
                if with_bias:
                    nc.tensor.matmul(
                        pp_slice,
                        ONES[0:1, st * 128:(st + 1) * 128],
                        BQKV[0:1, 512:768],
                        start=False, stop=True,
                    )

            def emit_proj_block(qc):
                """Q/K chains for qc (2 psS pair-tiles; qc0 uses psZ/psO
                singles for the first two so more chains fly while x
                streams), then V chains for seq-tiles 4qc..4qc+4."""
                specs = [("q", WQh, WQl, QT, 0), ("k", WKh, WKl, KT, 256)]
                if qc == 0:
                    for name, w_hi, w_lo, dst, boff in specs:
                        for hp, pool in ((0, psZ), (1, psO)):
                            pz = pool.tile([128, QW], F32, tag="z",
                                           name=f"{name}z{hp}")
                            qk_chain(pz[:], w_hi, w_lo, hp, 0, boff)
                            t = dst[hp]
                            nc.gpsimd.tensor_copy(t[:, 0, :], pz[:])
                else:
                    for name, w_hi, w_lo, dst, boff in specs:
                        pp = psS.tile([128, 2, QW], F32, tag="s",
                                      name=f"{name}p{qc}")
                        for hp in range(2):
                            qk_chain(pp[:, hp, :], w_hi, w_lo, hp, qc, boff)
                        for hp in range(2):
                            nc.gpsimd.tensor_copy(dst[hp][:, qc, :], pp[:, hp, :])
                for st2 in range(2 * qc, 2 * qc + 2):
                    pp = psS.tile([128, 2, QW], F32, tag="s", name=f"vp{st2}")
                    for i in range(2):
                        v_chain(pp[:, i, 0:256], st2 * 2 + i)
                    for i in range(2):
                        st = st2 * 2 + i
                        nc.vector.tensor_copy(
                            VB[:, st, :, 0:DH],
                            pp[:, i, 0:256].rearrange("p (h d) -> p h d", h=HLOC),
                        )
                        if st < 12:
                            nc.vector.tensor_copy(
                                V8[:, st // 2, st % 2, :, 0:DH],
                                pp[:, i, 0:256].rearrange("p (h d) -> p h d", h=HLOC),
                            )

            # ---- attention + output projection for one q-chunk ----
            def emit_attention(qc):
                ZT = [None, None]
                for h in range(HLOC):
                    hp, hh = h // 2, h % 2
                    hs = slice(hh * DH, (hh + 1) * DH)
                    zps = psZ.tile([128, QW], F32, tag="z", name=f"z{qc}_{h}")

                    def s_mm(dst, kt, q_lo, q_hi):
                        nc.tensor.matmul(
                            dst,
                            KT[hp][hs, kt // 4, (kt % 4) * 128:(kt % 4 + 1) * 128],
                            QT[hp][hs, qc, q_lo:q_hi],
                            start=True, stop=True,
                        )

                    # off-diagonal k-pairs: bf16 scores, fp8 P, DoubleRow Z
                    for m in range(2 * qc):
                        sps = psS.tile([128, 2, QW], F32, tag="s",
                                       name=f"s{qc}_{h}_{m}")
                        for i in range(2):
                            s_mm(sps[:, i, :], 2 * m + i, 0, QW)
                        pt8 = ppt8.tile([128, 2, QW], F8, tag="p8",
                                        name=f"p8_{qc}_{h}_{m}")
                        nc.scalar.activation(
                            pt8[:], sps[:], mybir.ActivationFunctionType.Exp,
                            scale=EXP_SCALE,
                        )
                        nc.tensor.matmul(
                            zps[0:DH + 1, :],
                            V8[:, m, :, h, :],
                            pt8[:],
                            start=(m == 0), stop=False,
                            perf_mode=DR,
                            skip_group_check=True,
                        )

                    # diagonal 4 k-tiles kt=4qc..4qc+3 (moff 0,128,256,384):
                    # pair A full-width; pair B packed into one 512 slot
                    ktA, ktB = 4 * qc, 4 * qc + 2
                    sA = psS.tile([128, 2, QW], F32, tag="s", name=f"sA{qc}_{h}")
                    s_mm(sA[:, 0, :], ktA, 0, QW)
                    s_mm(sA[:, 1, :], ktA + 1, 0, QW)
                    ptA = pptb.tile([128, 2, QW], BF16, tag="pb",
                                    name=f"pA{qc}_{h}")
                    nc.scalar.activation(
                        ptA[:], sA[:], mybir.ActivationFunctionType.Exp,
                        scale=EXP_SCALE,
                    )
                    sB = psS.tile([128, 2, QW], F32, tag="s", name=f"sB{qc}_{h}")
                    s_mm(sB[:, 0, 0:256], ktB, 256, QW)
                    s_mm(sB[:, 0, 256:QW], ktB + 1, 256, QW)
                    ptB = pptb.tile([128, 2, QW], BF16, tag="pb",
                                    name=f"pB{qc}_{h}")
                    nc.scalar.activation(
                        ptB[:, 0, :], sB[:, 0, :],
                        mybir.ActivationFunctionType.Exp, scale=EXP_SCALE,
                    )
                    # triangular masks on the 4 diagonal boundaries
                    nc.vector.tensor_mul(ptA[:, 0, 0:128], ptA[:, 0, 0:128], TRI[:])
                    nc.vector.tensor_mul(ptA[:, 1, 128:256], ptA[:, 1, 128:256], TRI[:])
                    nc.vector.tensor_mul(ptB[:, 0, 0:128], ptB[:, 0, 0:128], TRI[:])
                    nc.vector.tensor_mul(ptB[:, 0, 384:QW], ptB[:, 0, 384:QW], TRI[:])

                    def z_mm(v_st, pt_ap, q_lo, q_hi, start, stop):
                        nc.tensor.matmul(
                            zps[0:DH + 1, q_lo:q_hi],
                            VB[:, v_st, h, :],
                            pt_ap,
                            start=start, stop=stop,
                            skip_group_check=True,
                        )

                    z_mm(ktA, ptA[:, 0, :], 0, QW, qc == 0, False)
                    z_mm(ktA + 1, ptA[:, 1, 128:QW], 128, QW, False, False)
                    z_mm(ktB, ptB[:, 0, 0:256], 256, QW, False, False)
                    z_mm(ktB + 1, ptB[:, 0, 384:QW], 384, QW, False, True)

                    # normalize: ZT[hh*64:...] = zps[0:64] * (1/zps[64])
                    recip = prs.tile([1, QW], F32R, tag="recip",
                                     name=f"rc{qc}_{h}")
                    with nc.allow_low_precision(reason="softmax recip in fp32r"):
                        nc.vector.reciprocal(recip[:], zps[DH:DH + 1, :])
                    rb = prs.tile([DH, QW], F32R, tag="rb", name=f"rb{qc}_{h}")
                    nc.gpsimd.partition_broadcast(rb[:], recip[:])
                    if ZT[hp] is None:
                        ZT[hp] = pzt.tile([128, QW], F32R, tag="zt",
                                          name=f"zt{qc}_{hp}")
                    nc.vector.tensor_mul(ZT[hp][hs, :], zps[0:DH, :], rb[:])

                # out[q0:q0+512, :] = sum_hp ZT[hp].T @ WO[hp] (bf16 moving)
                q0 = qc * QW
                for qt in range(4):
                    osb = pout.tile([128, DM], F32, tag="ob", name=f"ob{qc}_{qt}")
                    for mc in range(2):
                        if qc == 3:
                            # tail: spread across all psum pools so the final
                            # 8 copies+DMAs overlap maximally
                            k = qt * 2 + mc
                            pool, shp = ((psO, [128, QW]), (psZ, [128, QW]),
                                         (psS, [128, 2, QW]))[k % 3]
                            t = pool.tile(shp, F32,
                                          tag="z" if pool is psZ else
                                          ("s" if pool is psS else "z"),
                                          name=f"o3_{k}")
                            ops = t[:, 0, :] if pool is psS else t[:]
                        else:
                            t = psO.tile([128, QW], F32, tag="z",
                                         name=f"o{qc}_{qt}_{mc}")
                            ops = t[:]
                        for hp in range(2):
                            nc.tensor.matmul(
                                ops,
                                ZT[hp][:, qt * 128:(qt + 1) * 128],
                                WO[:, hp, mc * QW:(mc + 1) * QW],
                                start=(hp == 0), stop=(hp == 1),
                            )
                        eng = nc.vector if (qt * 2 + mc) % 2 == 0 else nc.gpsimd
                        eng.tensor_copy(osb[:, mc * QW:(mc + 1) * QW], ops)
                        nc.sync.dma_start(
                            out[q0 + qt * 128:q0 + (qt + 1) * 128,
                                mc * QW:(mc + 1) * QW],
                            osb[:, mc * QW:(mc + 1) * QW],
                        )

            for qc in range(NQC):
                emit_proj_block(qc)
                emit_attention(qc)

    nc.compile()
    return nc


def _get_program(with_bias: bool):
    if with_bias not in _PROGRAMS:
        _PROGRAMS[with_bias] = _build(with_bias)
    return _PROGRAMS[with_bias]


def _split8(a):
    hi = a.astype(ml_dtypes.float8_e4m3)
    lo = (a - hi.astype(np.float32)).astype(ml_dtypes.float8_e4m3)
    return hi, lo


def _x_layout(a):
    # [1024, 2048] -> [128, j, i, qb, s]
    return np.ascontiguousarray(
        a.reshape(KCP, 2, 128, NQC, QW).transpose(2, 0, 1, 3, 4))


def _w_layout(a):
    # [1024, 256] -> [128, j, i, 256]
    return np.ascontiguousarray(
        a.reshape(KCP, 2, 128, 256).transpose(2, 0, 1, 3))


def kernel(normalized_resid_pre, W_Q, W_K, W_V, W_O, b_Q, b_K, b_V, b_O):
    x = np.asarray(normalized_resid_pre, dtype=np.float32)
    W_Q = np.asarray(W_Q, dtype=np.float32)
    W_K = np.asarray(W_K, dtype=np.float32)
    W_V = np.asarray(W_V, dtype=np.float32)
    W_O = np.asarray(W_O, dtype=np.float32)
    b_Q = np.asarray(b_Q, dtype=np.float32)
    b_K = np.asarray(b_K, dtype=np.float32)
    b_V = np.asarray(b_V, dtype=np.float32)
    b_O = np.asarray(b_O, dtype=np.float32)

    batch, seq, dm = x.shape
    with_bias = bool(np.any(b_Q) or np.any(b_K) or np.any(b_V))
    nc = _get_program(with_bias)

    tri = np.ascontiguousarray(
        np.triu(np.ones((128, 128), np.float32)).astype(ml_dtypes.bfloat16))

    xsp = []
    for b in range(batch):
        xh, xl = _split8(np.ascontiguousarray(x[b].T))
        xsp.append((_x_layout(xh), _x_layout(xl)))

    in_maps = []
    for c in range(8):
        b, g = c // 4, c % 4
        hs = slice(4 * g, 4 * g + 4)
        m = {"xh8": xsp[b][0], "xl8": xsp[b][1], "tri": tri}
        for nm, W in (("wq", W_Q), ("wk", W_K), ("wv", W_V)):
            Wp = np.transpose(W[hs], (1, 0, 2)).reshape(dm, 256) * WS
            hi, lo = _split8(Wp)
            m[nm + "h"] = _w_layout(hi)
            m[nm + "l"] = _w_layout(lo)
        m["wo"] = np.ascontiguousarray(
            (W_O[hs].reshape(256, dm) / WS).astype(ml_dtypes.bfloat16))
        if with_bias:
            m["bqkv"] = np.ascontiguousarray(np.concatenate(
                [b_Q[hs].reshape(256) * WS, b_K[hs].reshape(256) * WS,
                 b_V[hs].reshape(256) * WS]
            )[None, :].astype(ml_dtypes.bfloat16))
            m["ones"] = np.ones((1, seq), ml_dtypes.bfloat16)
        in_maps.append(m)

    res = bass_utils.run_bass_kernel_spmd(nc, in_maps, core_ids=list(range(8)))
    parts = [res.results[c]["out"] for c in range(8)]
    full = np.stack(
        [parts[0] + parts[1] + parts[2] + parts[3],
         parts[4] + parts[5] + parts[6] + parts[7]]
    )
    full += b_O
    return full.astype(np.float32)


# revision 6
# speedup vs baseline: 1.1517x; 1.1408x over previous
"""Multi-head causal attention (B=2, S=2048, D=1024, H=16, Dh=64) on 8 TRN2 cores.

Sharding: tensor-parallel over 4 head-groups x data-parallel over 2 batches.
Core c handles batch c//4, heads [4*(c%4), 4*(c%4)+4). Each core computes its
partial output projection; the host sums the 4 partials per batch (the
"all-reduce") and adds b_O.

Mixed-precision device program (fp32 PSUM accumulation everywhere):
  - Q/K/V projections: fp8e4m3 DoubleRow matmuls (0.5 cyc/row, 256-deep
    contraction per instr). Host pre-splits x and 64*W into hi/lo fp8 pairs;
    3 compensation terms xh@Wh + xl@Wh + xh@Wl recover ~bf16 accuracy.
  - Scores S = (64q)(64k): bf16 QT/KT, per-128-column causal crops. The /8
    softmax scale and the 1/4096 fp8 weight scaling fold into the ACT exp
    (scale=1/32768).
  - P = exp(S): ACT writes fp8 tiles for off-diagonal k-pairs, bf16 for the
    4 diagonal k-tiles (where P concentrates and fp8 noise would not average
    out). Triangular masks multiply bf16 diag tiles only (DVE 2x on bf16).
  - Z = P @ [64V|1]: off-diag via fp8 DoubleRow over k-tile pairs (0.25
    cyc/row/k-tile), diag via bf16-moving matmuls; row 64 accumulates the
    softmax denominator.
  - out += (Z/denom)^T @ (Wo/64): f32r stationary ZT, bf16 moving WO.
"""

import numpy as np
import ml_dtypes

import concourse.mybir as mybir
import concourse.tile as tile
from concourse import bacc
from concourse import bass_utils

F32 = mybir.dt.float32
F32R = mybir.dt.float32r
BF16 = mybir.dt.bfloat16
F8 = mybir.dt.float8e4

SEQ = 2048
DM = 1024
DH = 64
HLOC = 4          # heads per core
KCP = 4           # dmodel pair-chunks of 256
NQC = 4           # q chunks of 512
QW = 512
WS = 64.0         # fp8 weight scale
EXP_SCALE = 1.0 / (WS * WS * 8.0)
DR = mybir.MatmulPerfMode.DoubleRow

_PROGRAMS = {}


def _build(with_bias: bool):
    nc = bacc.Bacc("TRN2", target_bir_lowering=False, debug=False, num_devices=8)

    # [128, j(4), i(2), qb(4), 512] with element [p,j,i,qb,s] = xT[256j+128i+p,
    # 512qb+s]; hi/lo fp8 split of xT
    xh8 = nc.dram_tensor("xh8", [128, KCP, 2, NQC, QW], F8, kind="ExternalInput").ap()
    xl8 = nc.dram_tensor("xl8", [128, KCP, 2, NQC, QW], F8, kind="ExternalInput").ap()
    # [128, j(4), i(2), 256]: hi/lo of 64*W[256j+128i+p, m]
    wname = lambda n: nc.dram_tensor(n, [128, KCP, 2, 256], F8, kind="ExternalInput").ap()
    wqh, wql = wname("wqh"), wname("wql")
    wkh, wkl = wname("wkh"), wname("wkl")
    wvh, wvl = wname("wvh"), wname("wvl")
    wo = nc.dram_tensor("wo", [256, DM], BF16, kind="ExternalInput").ap()
    tri = nc.dram_tensor("tri", [128, 128], BF16, kind="ExternalInput").ap()
    if with_bias:
        bqkv = nc.dram_tensor("bqkv", [1, 768], BF16, kind="ExternalInput").ap()
        ones = nc.dram_tensor("ones", [1, SEQ], BF16, kind="ExternalInput").ap()
    out = nc.dram_tensor("out", [SEQ, DM], F32, kind="ExternalOutput").ap()

    with tile.TileContext(nc) as tc:
        with (
            tc.tile_pool(name="px", bufs=1) as px,
            tc.tile_pool(name="pw", bufs=1) as pw,
            tc.tile_pool(name="pqk", bufs=1) as pqk,
            tc.tile_pool(name="pv", bufs=1) as pv,
            tc.tile_pool(name="ppt8", bufs=4) as ppt8,
            tc.tile_pool(name="pptb", bufs=4) as pptb,
            tc.tile_pool(name="pzt", bufs=4) as pzt,
            tc.tile_pool(name="prs", bufs=3) as prs,
            tc.tile_pool(name="pout", bufs=4) as pout,
            tc.tile_pool(name="psS", bufs=3, space="PSUM") as psS,
            tc.tile_pool(name="psZ", bufs=2, space="PSUM") as psZ,
        ):
            # ---- SBUF tiles ----
            XH = px.tile([128, KCP, 2, NQC, QW], F8, tag="xh")
            XL = px.tile([128, KCP, 2, NQC, QW], F8, tag="xl")
            WQh = pw.tile([128, KCP, 2, 256], F8, tag="wqh")
            WQl = pw.tile([128, KCP, 2, 256], F8, tag="wql")
            WKh = pw.tile([128, KCP, 2, 256], F8, tag="wkh")
            WKl = pw.tile([128, KCP, 2, 256], F8, tag="wkl")
            WVh = pw.tile([128, KCP, 2, 256], F8, tag="wvh")
            WVl = pw.tile([128, KCP, 2, 256], F8, tag="wvl")

            # ---- input DMA: qb0 x-blocks + QK weights first ----
            nc.sync.dma_start(WQh[:], wqh)
            nc.sync.dma_start(WQl[:], wql)
            for j in range(KCP):
                nc.sync.dma_start(XH[:, j, :, 0, :], xh8[:, j, :, 0, :])
                nc.sync.dma_start(XL[:, j, :, 0, :], xl8[:, j, :, 0, :])
            nc.sync.dma_start(WKh[:], wkh)
            nc.sync.dma_start(WKl[:], wkl)
            nc.sync.dma_start(WVh[:], wvh)
            nc.sync.dma_start(WVl[:], wvl)
            TRI = pw.tile([128, 128], BF16, tag="tri")
            nc.sync.dma_start(TRI[:], tri)
            for j in range(KCP):
                nc.sync.dma_start(XH[:, j, :, 1, :], xh8[:, j, :, 1, :])
                nc.sync.dma_start(XL[:, j, :, 1, :], xl8[:, j, :, 1, :])
            WO = pw.tile([128, 2, DM], BF16, tag="wo")
            for hp in range(2):
                nc.sync.dma_start(WO[:, hp, :], wo[hp * 128:(hp + 1) * 128, :])
            for qb in range(2, NQC):
                for j in range(KCP):
                    nc.sync.dma_start(XH[:, j, :, qb, :], xh8[:, j, :, qb, :])
                    nc.sync.dma_start(XL[:, j, :, qb, :], xl8[:, j, :, qb, :])
            if with_bias:
                BQKV = pw.tile([1, 768], BF16, tag="bqkv")
                nc.sync.dma_start(BQKV[:], bqkv)
                ONES = pw.tile([1, SEQ], BF16, tag="ones")
                nc.sync.dma_start(ONES[:], ones)

            # QT/KT bf16 [128 (head-pair), qc, 512] per hp; VB f32r with ones
            # col; V8 fp8 pairs (st=2m+i, only m<6 used off-diagonal)
            QT = [pqk.tile([128, NQC, QW], BF16, tag=f"qt{hp}", name=f"qt{hp}")
                  for hp in range(2)]
            KT = [pqk.tile([128, NQC, QW], BF16, tag=f"kt{hp}", name=f"kt{hp}")
                  for hp in range(2)]
            VB = pv.tile([128, 16, HLOC, DH + 1], F32R, tag="vb")
            V8 = pv.tile([128, 6, 2, HLOC, DH + 1], F8, tag="v8")
            nc.vector.memset(VB[:, :, :, DH:DH + 1], 1.0)
            nc.vector.memset(V8[:, :, :, :, DH:DH + 1], 1.0)

            # ---- projections: fp8 DoubleRow 3-term chains ----
            def qk_chain(pp_slice, w_hi, w_lo, hp, qc, bias_off):
                for j in range(KCP):
                    for term, (w_sb, x_sb) in enumerate(
                        ((w_hi, XH), (w_hi, XL), (w_lo, XH))
                    ):
                        nc.tensor.matmul(
                            pp_slice,
                            w_sb[:, j, :, hp * 128:(hp + 1) * 128],
                            x_sb[:, j, :, qc, :],
                            start=(j == 0 and term == 0),
                            stop=(j == KCP - 1 and term == 2 and not with_bias),
                            perf_mode=DR,
                        )
                if with_bias:
                    nc.tensor.matmul(
                        pp_slice,
                        BQKV[0:1, bias_off + hp * 128:bias_off + (hp + 1) * 128],
                        ONES[0:1, qc * QW:(qc + 1) * QW],
                        start=False, stop=True,
                    )

            def v_chain(pp_slice, st):
                qb, s0 = st // 4, (st % 4) * 128
                for j in range(KCP):
                    for term, (x_sb, w_sb) in enumerate(
                        ((XH, WVh), (XL, WVh), (XH, WVl))
                    ):
                        nc.tensor.matmul(
                            pp_slice,
                            x_sb[:, j, :, qb, s0:s0 + 128],
                            w_sb[:, j, :, :],
                            start=(j == 0 and term == 0),
                            stop=(j == KCP - 1 and term == 2 and not with_bias),
                            perf_mode=DR,
                        )
                if with_bias:
                    nc.tensor.matmul(
                        pp_slice,
                        ONES[0:1, st * 128:(st + 1) * 128],
                        BQKV[0:1, 512:768],
                        start=False, stop=True,
                    )

            # ---- projection chain-tile emitters (PE filler units) ----
            def emit_qk_singles0():
                # qc0 Q chains as psZ singles (psZ idle until attention)
                for hp in range(2):
                    pz = psZ.tile([128, QW], F32, tag="z", name=f"qz{hp}")
                    qk_chain(pz[:], WQh, WQl, hp, 0, 0)
                    nc.gpsimd.tensor_copy(QT[hp][:, 0, :], pz[:])

            def emit_qk_pair(qc, which):
                w_hi, w_lo, dst, boff = (
                    (WQh, WQl, QT, 0) if which == "q" else (WKh, WKl, KT, 256))
                pp = psS.tile([128, 2, QW], F32, tag="s", name=f"{which}p{qc}")
                for hp in range(2):
                    qk_chain(pp[:, hp, :], w_hi, w_lo, hp, qc, boff)
                for hp in range(2):
                    nc.gpsimd.tensor_copy(dst[hp][:, qc, :], pp[:, hp, :])

            def emit_v_pair(st2):
                pp = psS.tile([128, 2, QW], F32, tag="s", name=f"vp{st2}")
                for i in range(2):
                    v_chain(pp[:, i, 0:256], st2 * 2 + i)
                for i in range(2):
                    st = st2 * 2 + i
                    nc.vector.tensor_copy(
                        VB[:, st, :, 0:DH],
                        pp[:, i, 0:256].rearrange("p (h d) -> p h d", h=HLOC),
                    )
                    if st < 12:
                        nc.vector.tensor_copy(
                            V8[:, st // 2, st % 2, :, 0:DH],
                            pp[:, i, 0:256].rearrange("p (h d) -> p h d", h=HLOC),
                        )

            # ---- output projection units (PE filler; 2 (qt,mc) per tile) ----
            ZTS = {}   # qc -> [ZT_hp0, ZT_hp1]

            def emit_o_unit(qc, u):
                # u in 0..3: qt pair-of-(mc) unit -> one psS tile
                q0 = qc * QW
                qt = u
                ZT = ZTS[qc]
                t = psS.tile([128, 2, QW], F32, tag="s", name=f"o{qc}_{u}")
                osb = pout.tile([128, DM], F32, tag="ob", name=f"ob{qc}_{qt}")
                for mc in range(2):
                    for hp in range(2):
                        nc.tensor.matmul(
                            t[:, mc, :],
                            ZT[hp][:, qt * 128:(qt + 1) * 128],
                            WO[:, hp, mc * QW:(mc + 1) * QW],
                            start=(hp == 0), stop=(hp == 1),
                        )
                for mc in range(2):
                    eng = nc.vector if (qt + mc) % 2 == 0 else nc.gpsimd
                    eng.tensor_copy(osb[:, mc * QW:(mc + 1) * QW], t[:, mc, :])
                    nc.sync.dma_start(
                        out[q0 + qt * 128:q0 + (qt + 1) * 128,
                            mc * QW:(mc + 1) * QW],
                        osb[:, mc * QW:(mc + 1) * QW],
                    )

            # ---- one attention head: scores->exp->Z with the Z matmuls
            # delayed one s-tile behind (exp latency hiding) and a PE filler
            # emitted before the diagonal Z drain ----
            def emit_head(qc, h, fillers):
                hp, hh = h // 2, h % 2
                hs = slice(hh * DH, (hh + 1) * DH)
                zps = psZ.tile([128, QW], F32, tag="z", name=f"z{qc}_{h}")

                def s_mm(dst, kt, q_lo, q_hi):
                    nc.tensor.matmul(
                        dst,
                        KT[hp][hs, kt // 4, (kt % 4) * 128:(kt % 4 + 1) * 128],
                        QT[hp][hs, qc, q_lo:q_hi],
                        start=True, stop=True,
                    )

                def z_dr(m, pt8):
                    nc.tensor.matmul(
                        zps[0:DH + 1, :], V8[:, m, :, h, :], pt8[:],
                        start=(m == 0), stop=False,
                        perf_mode=DR, skip_group_check=True,
                    )

                pend = []
                for m in range(2 * qc):
                    sps = psS.tile([128, 2, QW], F32, tag="s",
                                   name=f"s{qc}_{h}_{m}")
                    for i in range(2):
                        s_mm(sps[:, i, :], 2 * m + i, 0, QW)
                    pt8 = ppt8.tile([128, 2, QW], F8, tag="p8",
                                    name=f"p8_{qc}_{h}_{m}")
                    nc.scalar.activation(
                        pt8[:], sps[:], mybir.ActivationFunctionType.Exp,
                        scale=EXP_SCALE,
                    )
                    pend.append((z_dr, m, pt8))
                    if len(pend) > 1:
                        fn, mm, pt = pend.pop(0)
                        fn(mm, pt)

                ktA, ktB = 4 * qc, 4 * qc + 2
                sA = psS.tile([128, 2, QW], F32, tag="s", name=f"sA{qc}_{h}")
                s_mm(sA[:, 0, :], ktA, 0, QW)
                s_mm(sA[:, 1, :], ktA + 1, 0, QW)
                ptA = pptb.tile([128, 2, QW], BF16, tag="pb", name=f"pA{qc}_{h}")
                nc.scalar.activation(
                    ptA[:], sA[:], mybir.ActivationFunctionType.Exp,
                    scale=EXP_SCALE,
                )
                for fn, mm, pt in pend:
                    fn(mm, pt)
                sB = psS.tile([128, 2, QW], F32, tag="s", name=f"sB{qc}_{h}")
                s_mm(sB[:, 0, 0:256], ktB, 256, QW)
                s_mm(sB[:, 0, 256:QW], ktB + 1, 256, QW)
                ptB = pptb.tile([128, 2, QW], BF16, tag="pb", name=f"pB{qc}_{h}")
                nc.scalar.activation(
                    ptB[:, 0, :], sB[:, 0, :],
                    mybir.ActivationFunctionType.Exp, scale=EXP_SCALE,
                )
                nc.vector.tensor_mul(ptA[:, 0, 0:128], ptA[:, 0, 0:128], TRI[:])
                nc.vector.tensor_mul(ptA[:, 1, 128:256], ptA[:, 1, 128:256], TRI[:])
                nc.vector.tensor_mul(ptB[:, 0, 0:128], ptB[:, 0, 0:128], TRI[:])
                nc.vector.tensor_mul(ptB[:, 0, 384:QW], ptB[:, 0, 384:QW], TRI[:])

                # PE filler while the diagonal exps+masks complete
                for f in fillers:
                    f()

                def z_bf(v_st, pt_ap, q_lo, q_hi, start, stop):
                    nc.tensor.matmul(
                        zps[0:DH + 1, q_lo:q_hi],
                        VB[:, v_st, h, :], pt_ap,
                        start=start, stop=stop, skip_group_check=True,
                    )

                z_bf(ktA, ptA[:, 0, :], 0, QW, qc == 0, False)
                z_bf(ktA + 1, ptA[:, 1, 128:QW], 128, QW, False, False)
                z_bf(ktB, ptB[:, 0, 0:256], 256, QW, False, False)
                z_bf(ktB + 1, ptB[:, 0, 384:QW], 384, QW, False, True)

                recip = prs.tile([1, QW], F32R, tag="recip", name=f"rc{qc}_{h}")
                with nc.allow_low_precision(reason="softmax recip in fp32r"):
                    nc.vector.reciprocal(recip[:], zps[DH:DH + 1, :])
                rb = prs.tile([DH, QW], F32R, tag="rb", name=f"rb{qc}_{h}")
                nc.gpsimd.partition_broadcast(rb[:], recip[:])
                if qc not in ZTS:
                    ZTS[qc] = [
                        pzt.tile([128, QW], F32R, tag="zt", name=f"zt{qc}_{p}")
                        for p in range(2)
                    ]
                nc.vector.tensor_mul(ZTS[qc][hp][hs, :], zps[0:DH, :], rb[:])

            # ---- global pipelined emission ----
            # proj(0): Q singles on psZ, K pair + V pairs on psS
            emit_qk_singles0()
            emit_qk_pair(0, "k")
            emit_v_pair(0)
            emit_v_pair(1)
            # head stream with PE fillers: proj(qc+1) during attention(qc),
            # O-proj(qc-1) during attention(qc+1)
            emit_head(0, 0, [lambda: emit_qk_pair(1, "q")])
            emit_head(0, 1, [lambda: emit_qk_pair(1, "k")])
            emit_head(0, 2, [lambda: emit_v_pair(2)])
            emit_head(0, 3, [lambda: emit_v_pair(3)])
            emit_head(1, 0, [lambda: emit_qk_pair(2, "q")])
            emit_head(1, 1, [lambda: emit_qk_pair(2, "k"),
                             lambda: emit_o_unit(0, 0)])
            emit_head(1, 2, [lambda: emit_v_pair(4),
                             lambda: emit_o_unit(0, 1)])
            emit_head(1, 3, [lambda: emit_v_pair(5),
                             lambda: emit_o_unit(0, 2)])
            emit_head(2, 0, [lambda: emit_qk_pair(3, "q"),
                             lambda: emit_o_unit(0, 3)])
            emit_head(2, 1, [lambda: emit_qk_pair(3, "k"),
                             lambda: emit_o_unit(1, 0)])
            emit_head(2, 2, [lambda: emit_v_pair(6),
                             lambda: emit_o_unit(1, 1)])
            emit_head(2, 3, [lambda: emit_v_pair(7),
                             lambda: emit_o_unit(1, 2)])
            emit_head(3, 0, [lambda: emit_o_unit(1, 3)])
            emit_head(3, 1, [lambda: emit_o_unit(2, 0),
                             lambda: emit_o_unit(2, 1)])
            emit_head(3, 2, [lambda: emit_o_unit(2, 2)])
            emit_head(3, 3, [lambda: emit_o_unit(2, 3)])
            for u in range(4):
                emit_o_unit(3, u)

    nc.compile()
    return nc


def _get_program(with_bias: bool):
    if with_bias not in _PROGRAMS:
        _PROGRAMS[with_bias] = _build(with_bias)
    return _PROGRAMS[with_bias]


def _split8(a):
    hi = a.astype(ml_dtypes.float8_e4m3)
    lo = (a - hi.astype(np.float32)).astype(ml_dtypes.float8_e4m3)
    return hi, lo


def _x_layout(a):
    # [1024, 2048] -> [128, j, i, qb, s]
    return np.ascontiguousarray(
        a.reshape(KCP, 2, 128, NQC, QW).transpose(2, 0, 1, 3, 4))


def _w_layout(a):
    # [1024, 256] -> [128, j, i, 256]
    return np.ascontiguousarray(
        a.reshape(KCP, 2, 128, 256).transpose(2, 0, 1, 3))


def kernel(normalized_resid_pre, W_Q, W_K, W_V, W_O, b_Q, b_K, b_V, b_O):
    x = np.asarray(normalized_resid_pre, dtype=np.float32)
    W_Q = np.asarray(W_Q, dtype=np.float32)
    W_K = np.asarray(W_K, dtype=np.float32)
    W_V = np.asarray(W_V, dtype=np.float32)
    W_O = np.asarray(W_O, dtype=np.float32)
    b_Q = np.asarray(b_Q, dtype=np.float32)
    b_K = np.asarray(b_K, dtype=np.float32)
    b_V = np.asarray(b_V, dtype=np.float32)
    b_O = np.asarray(b_O, dtype=np.float32)

    batch, seq, dm = x.shape
    with_bias = bool(np.any(b_Q) or np.any(b_K) or np.any(b_V))
    nc = _get_program(with_bias)

    tri = np.ascontiguousarray(
        np.triu(np.ones((128, 128), np.float32)).astype(ml_dtypes.bfloat16))

    xsp = []
    for b in range(batch):
        xh, xl = _split8(np.ascontiguousarray(x[b].T))
        xsp.append((_x_layout(xh), _x_layout(xl)))

    in_maps = []
    for c in range(8):
        b, g = c // 4, c % 4
        hs = slice(4 * g, 4 * g + 4)
        m = {"xh8": xsp[b][0], "xl8": xsp[b][1], "tri": tri}
        for nm, W in (("wq", W_Q), ("wk", W_K), ("wv", W_V)):
            Wp = np.transpose(W[hs], (1, 0, 2)).reshape(dm, 256) * WS
            hi, lo = _split8(Wp)
            m[nm + "h"] = _w_layout(hi)
            m[nm + "l"] = _w_layout(lo)
        m["wo"] = np.ascontiguousarray(
            (W_O[hs].reshape(256, dm) / WS).astype(ml_dtypes.bfloat16))
        if with_bias:
            m["bqkv"] = np.ascontiguousarray(np.concatenate(
                [b_Q[hs].reshape(256) * WS, b_K[hs].reshape(256) * WS,
                 b_V[hs].reshape(256) * WS]
            )[None, :].astype(ml_dtypes.bfloat16))
            m["ones"] = np.ones((1, seq), ml_dtypes.bfloat16)
        in_maps.append(m)

    res = bass_utils.run_bass_kernel_spmd(nc, in_maps, core_ids=list(range(8)))
    parts = [res.results[c]["out"] for c in range(8)]
    full = np.stack(
        [parts[0] + parts[1] + parts[2] + parts[3],
         parts[4] + parts[5] + parts[6] + parts[7]]
    )
    full += b_O
    return full.astype(np.float32)


# revision 9
# speedup vs baseline: 1.1577x; 1.0052x over previous
"""Multi-head causal attention (B=2, S=2048, D=1024, H=16, Dh=64) on 8 TRN2 cores.

Sharding: tensor-parallel over 4 head-groups x data-parallel over 2 batches.
Core c handles batch c//4, heads [4*(c%4), 4*(c%4)+4). Each core computes its
partial output projection; the host sums the 4 partials per batch (the
"all-reduce") and adds b_O.

Mixed-precision device program (fp32 PSUM accumulation everywhere):
  - Q/K/V projections: fp8e4m3 DoubleRow matmuls (0.5 cyc/row, 256-deep
    contraction per instr). Host pre-splits x and 64*W into hi/lo fp8 pairs;
    3 compensation terms xh@Wh + xl@Wh + xh@Wl recover ~bf16 accuracy.
  - Scores S = (64q)(64k): bf16 QT/KT, per-128-column causal crops. The /8
    softmax scale and the 1/4096 fp8 weight scaling fold into the ACT exp
    (scale=1/32768).
  - P = exp(S): ACT writes fp8 tiles for off-diagonal k-pairs, bf16 for the
    4 diagonal k-tiles (where P concentrates and fp8 noise would not average
    out). Triangular masks multiply bf16 diag tiles only (DVE 2x on bf16).
  - Z = P @ [64V|1]: off-diag via fp8 DoubleRow over k-tile pairs (0.25
    cyc/row/k-tile), diag via bf16-moving matmuls; row 64 accumulates the
    softmax denominator.
  - out += (Z/denom)^T @ (Wo/64): f32r stationary ZT, bf16 moving WO.
"""

import numpy as np
import ml_dtypes

import concourse.mybir as mybir
import concourse.tile as tile
from concourse import bacc
from concourse import bass_utils

F32 = mybir.dt.float32
F32R = mybir.dt.float32r
BF16 = mybir.dt.bfloat16
F8 = mybir.dt.float8e4

SEQ = 2048
DM = 1024
DH = 64
HLOC = 4          # heads per core
KCP = 4           # dmodel pair-chunks of 256
NQC = 4           # q chunks of 512
QW = 512
WS = 64.0         # fp8 weight scale
EXP_SCALE = 1.0 / (WS * WS * 8.0)
DR = mybir.MatmulPerfMode.DoubleRow

_PROGRAMS = {}


def _build(with_bias: bool):
    nc = bacc.Bacc("TRN2", target_bir_lowering=False, debug=False, num_devices=8)

    # [128, j(4), i(2), qb(4), 512] with element [p,j,i,qb,s] = xT[256j+128i+p,
    # 512qb+s]; hi/lo fp8 split of xT
    xh8 = nc.dram_tensor("xh8", [128, KCP, 2, NQC, QW], F8, kind="ExternalInput").ap()
    xl8 = nc.dram_tensor("xl8", [128, KCP, 2, NQC, QW], F8, kind="ExternalInput").ap()
    # [128, j(4), i(2), 256]: hi/lo of 64*W[256j+128i+p, m]
    wname = lambda n: nc.dram_tensor(n, [128, KCP, 2, 256], F8, kind="ExternalInput").ap()
    wqh, wql = wname("wqh"), wname("wql")
    wkh, wkl = wname("wkh"), wname("wkl")
    wvh, wvl = wname("wvh"), wname("wvl")
    wo = nc.dram_tensor("wo", [256, DM], BF16, kind="ExternalInput").ap()
    tri = nc.dram_tensor("tri", [128, 128], BF16, kind="ExternalInput").ap()
    if with_bias:
        bqkv = nc.dram_tensor("bqkv", [1, 768], BF16, kind="ExternalInput").ap()
        ones = nc.dram_tensor("ones", [1, SEQ], BF16, kind="ExternalInput").ap()
    out = nc.dram_tensor("out", [SEQ, DM], F32, kind="ExternalOutput").ap()

    with tile.TileContext(nc) as tc:
        with (
            tc.tile_pool(name="px", bufs=1) as px,
            tc.tile_pool(name="pw", bufs=1) as pw,
            tc.tile_pool(name="pqk", bufs=1) as pqk,
            tc.tile_pool(name="pv", bufs=1) as pv,
            tc.tile_pool(name="ppt8", bufs=4) as ppt8,
            tc.tile_pool(name="pptb", bufs=4) as pptb,
            tc.tile_pool(name="pzt", bufs=4) as pzt,
            tc.tile_pool(name="prs", bufs=3) as prs,
            tc.tile_pool(name="pout", bufs=4) as pout,
            tc.tile_pool(name="psS", bufs=3, space="PSUM") as psS,
            tc.tile_pool(name="psZ", bufs=2, space="PSUM") as psZ,
        ):
            # ---- SBUF tiles ----
            XH = px.tile([128, KCP, 2, NQC, QW], F8, tag="xh")
            XL = px.tile([128, KCP, 2, NQC, QW], F8, tag="xl")
            WQh = pw.tile([128, KCP, 2, 256], F8, tag="wqh")
            WQl = pw.tile([128, KCP, 2, 256], F8, tag="wql")
            WKh = pw.tile([128, KCP, 2, 256], F8, tag="wkh")
            WKl = pw.tile([128, KCP, 2, 256], F8, tag="wkl")
            WVh = pw.tile([128, KCP, 2, 256], F8, tag="wvh")
            WVl = pw.tile([128, KCP, 2, 256], F8, tag="wvl")

            # ---- input DMA: qb0 x-blocks + QK weights first ----
            nc.sync.dma_start(WQh[:], wqh)
            nc.sync.dma_start(WQl[:], wql)
            nc.sync.dma_start(XH[:, :, :, 0, :], xh8[:, :, :, 0, :])
            nc.sync.dma_start(XL[:, :, :, 0, :], xl8[:, :, :, 0, :])
            nc.sync.dma_start(WKh[:], wkh)
            nc.sync.dma_start(WKl[:], wkl)
            nc.sync.dma_start(WVh[:], wvh)
            nc.sync.dma_start(WVl[:], wvl)
            TRI = pw.tile([128, 128], BF16, tag="tri")
            nc.sync.dma_start(TRI[:], tri)
            nc.sync.dma_start(XH[:, :, :, 1, :], xh8[:, :, :, 1, :])
            nc.sync.dma_start(XL[:, :, :, 1, :], xl8[:, :, :, 1, :])
            WO = pw.tile([128, 2, DM], BF16, tag="wo")
            for hp in range(2):
                nc.sync.dma_start(WO[:, hp, :], wo[hp * 128:(hp + 1) * 128, :])
            for qb in range(2, NQC):
                nc.sync.dma_start(XH[:, :, :, qb, :], xh8[:, :, :, qb, :])
                nc.sync.dma_start(XL[:, :, :, qb, :], xl8[:, :, :, qb, :])
            if with_bias:
                BQKV = pw.tile([1, 768], BF16, tag="bqkv")
                nc.sync.dma_start(BQKV[:], bqkv)
                ONES = pw.tile([1, SEQ], BF16, tag="ones")
                nc.sync.dma_start(ONES[:], ones)

            # QT/KT bf16 [128 (head-pair), qc, 512] per hp; VB f32r with ones
            # col; V8 fp8 pairs (st=2m+i, only m<6 used off-diagonal)
            QT = [pqk.tile([128, NQC, QW], BF16, tag=f"qt{hp}", name=f"qt{hp}")
                  for hp in range(2)]
            KT = [pqk.tile([128, NQC, QW], BF16, tag=f"kt{hp}", name=f"kt{hp}")
                  for hp in range(2)]
            VB = pv.tile([128, 16, HLOC, DH + 1], F32R, tag="vb")
            V8 = pv.tile([128, 6, 2, HLOC, DH + 1], F8, tag="v8")
            nc.vector.memset(VB[:, :, :, DH:DH + 1], 1.0)
            nc.vector.memset(V8[:, :, :, :, DH:DH + 1], 1.0)
            ones64 = pw.tile([1, DH], F32R, tag="ones64")
            nc.vector.memset(ones64[:], 1.0)

            # ---- projections: fp8 DoubleRow 3-term chains ----
            def qk_chain(pp_slice, w_hi, w_lo, hp, qc, bias_off):
                for j in range(KCP):
                    for term, (w_sb, x_sb) in enumerate(
                        ((w_hi, XH), (w_hi, XL), (w_lo, XH))
                    ):
                        nc.tensor.matmul(
                            pp_slice,
                            w_sb[:, j, :, hp * 128:(hp + 1) * 128],
                            x_sb[:, j, :, qc, :],
                            start=(j == 0 and term == 0),
                            stop=(j == KCP - 1 and term == 2 and not with_bias),
                            perf_mode=DR,
                        )
                if with_bias:
                    nc.tensor.matmul(
                        pp_slice,
                        BQKV[0:1, bias_off + hp * 128:bias_off + (hp + 1) * 128],
                        ONES[0:1, qc * QW:(qc + 1) * QW],
                        start=False, stop=True,
                    )

            def v_chain(pp_slice, st):
                qb, s0 = st // 4, (st % 4) * 128
                for j in range(KCP):
                    for term, (x_sb, w_sb) in enumerate(
                        ((XH, WVh), (XL, WVh), (XH, WVl))
                    ):
                        nc.tensor.matmul(
                            pp_slice,
                            x_sb[:, j, :, qb, s0:s0 + 128],
                            w_sb[:, j, :, :],
                            start=(j == 0 and term == 0),
                            stop=(j == KCP - 1 and term == 2 and not with_bias),
                            perf_mode=DR,
                        )
                if with_bias:
                    nc.tensor.matmul(
                        pp_slice,
                        ONES[0:1, st * 128:(st + 1) * 128],
                        BQKV[0:1, 512:768],
                        start=False, stop=True,
                    )

            # ---- projection chain-tile emitters (PE filler units) ----
            def emit_qk_singles0():
                # qc0 Q chains as psZ singles (psZ idle until attention)
                for hp in range(2):
                    pz = psZ.tile([128, QW], F32, tag="z", name=f"qz{hp}")
                    qk_chain(pz[:], WQh, WQl, hp, 0, 0)
                    nc.gpsimd.tensor_copy(QT[hp][:, 0, :], pz[:])

            def emit_qk_pair(qc, which):
                w_hi, w_lo, dst, boff = (
                    (WQh, WQl, QT, 0) if which == "q" else (WKh, WKl, KT, 256))
                pp = psS.tile([128, 2, QW], F32, tag="s", name=f"{which}p{qc}")
                for hp in range(2):
                    qk_chain(pp[:, hp, :], w_hi, w_lo, hp, qc, boff)
                for hp in range(2):
                    nc.gpsimd.tensor_copy(dst[hp][:, qc, :], pp[:, hp, :])

            def emit_v_pair(st2):
                pp = psS.tile([128, 2, QW], F32, tag="s", name=f"vp{st2}")
                for i in range(2):
                    v_chain(pp[:, i, 0:256], st2 * 2 + i)
                for i in range(2):
                    st = st2 * 2 + i
                    nc.vector.tensor_copy(
                        VB[:, st, :, 0:DH],
                        pp[:, i, 0:256].rearrange("p (h d) -> p h d", h=HLOC),
                    )
                    if st < 12:
                        nc.vector.tensor_copy(
                            V8[:, st // 2, st % 2, :, 0:DH],
                            pp[:, i, 0:256].rearrange("p (h d) -> p h d", h=HLOC),
                        )

            # ---- output projection units (PE filler; 2 (qt,mc) per tile) ----
            ZTS = {}   # qc -> [ZT_hp0, ZT_hp1]

            def emit_o_unit(qc, u):
                # u in 0..3: qt pair-of-(mc) unit -> one psS tile
                q0 = qc * QW
                qt = u
                ZT = ZTS[qc]
                t = psS.tile([128, 2, QW], F32, tag="s", name=f"o{qc}_{u}")
                osb = pout.tile([128, DM], F32, tag="ob", name=f"ob{qc}_{qt}")
                for mc in range(2):
                    for hp in range(2):
                        nc.tensor.matmul(
                            t[:, mc, :],
                            ZT[hp][:, qt * 128:(qt + 1) * 128],
                            WO[:, hp, mc * QW:(mc + 1) * QW],
                            start=(hp == 0), stop=(hp == 1),
                        )
                for mc in range(2):
                    eng = nc.vector if (qt + mc) % 2 == 0 else nc.gpsimd
                    eng.tensor_copy(osb[:, mc * QW:(mc + 1) * QW], t[:, mc, :])
                    nc.sync.dma_start(
                        out[q0 + qt * 128:q0 + (qt + 1) * 128,
                            mc * QW:(mc + 1) * QW],
                        osb[:, mc * QW:(mc + 1) * QW],
                    )

            # ---- one attention head: scores->exp->Z with the Z matmuls
            # delayed one s-tile behind (exp latency hiding) and a PE filler
            # emitted before the diagonal Z drain ----
            def emit_head(qc, h, fillers):
                hp, hh = h // 2, h % 2
                hs = slice(hh * DH, (hh + 1) * DH)
                zps = psZ.tile([128, QW], F32, tag="z", name=f"z{qc}_{h}")

                def s_mm(dst, kt, q_lo, q_hi):
                    nc.tensor.matmul(
                        dst,
                        KT[hp][hs, kt // 4, (kt % 4) * 128:(kt % 4 + 1) * 128],
                        QT[hp][hs, qc, q_lo:q_hi],
                        start=True, stop=True,
                    )

                def z_dr(m, pt8):
                    nc.tensor.matmul(
                        zps[0:DH + 1, :], V8[:, m, :, h, :], pt8[:],
                        start=(m == 0), stop=False,
                        perf_mode=DR, skip_group_check=True,
                    )

                pend = []
                for m in range(2 * qc):
                    sps = psS.tile([128, 2, QW], F32, tag="s",
                                   name=f"s{qc}_{h}_{m}")
                    for i in range(2):
                        s_mm(sps[:, i, :], 2 * m + i, 0, QW)
                    pt8 = ppt8.tile([128, 2, QW], F8, tag="p8",
                                    name=f"p8_{qc}_{h}_{m}")
                    nc.scalar.activation(
                        pt8[:], sps[:], mybir.ActivationFunctionType.Exp,
                        scale=EXP_SCALE,
                    )
                    pend.append((z_dr, m, pt8))
                    if len(pend) > 1:
                        fn, mm, pt = pend.pop(0)
                        fn(mm, pt)

                ktA, ktB = 4 * qc, 4 * qc + 2
                sA = psS.tile([128, 2, QW], F32, tag="s", name=f"sA{qc}_{h}")
                s_mm(sA[:, 0, :], ktA, 0, QW)
                s_mm(sA[:, 1, :], ktA + 1, 0, QW)
                ptA = pptb.tile([128, 2, QW], BF16, tag="pb", name=f"pA{qc}_{h}")
                nc.scalar.activation(
                    ptA[:], sA[:], mybir.ActivationFunctionType.Exp,
                    scale=EXP_SCALE,
                )
                for fn, mm, pt in pend:
                    fn(mm, pt)
                sB = psS.tile([128, 2, QW], F32, tag="s", name=f"sB{qc}_{h}")
                s_mm(sB[:, 0, 0:256], ktB, 256, QW)
                s_mm(sB[:, 0, 256:QW], ktB + 1, 256, QW)
                ptB = pptb.tile([128, 2, QW], BF16, tag="pb", name=f"pB{qc}_{h}")
                nc.scalar.activation(
                    ptB[:, 0, :], sB[:, 0, :],
                    mybir.ActivationFunctionType.Exp, scale=EXP_SCALE,
                )
                nc.vector.tensor_mul(ptA[:, 0, 0:128], ptA[:, 0, 0:128], TRI[:])
                nc.vector.tensor_mul(ptA[:, 1, 128:256], ptA[:, 1, 128:256], TRI[:])
                nc.vector.tensor_mul(ptB[:, 0, 0:128], ptB[:, 0, 0:128], TRI[:])
                nc.vector.tensor_mul(ptB[:, 0, 384:QW], ptB[:, 0, 384:QW], TRI[:])

                # PE filler while the diagonal exps+masks complete
                for f in fillers:
                    f()

                def z_bf(v_st, pt_ap, q_lo, q_hi, start, stop):
                    nc.tensor.matmul(
                        zps[0:DH + 1, q_lo:q_hi],
                        VB[:, v_st, h, :], pt_ap,
                        start=start, stop=stop, skip_group_check=True,
                    )

                z_bf(ktA, ptA[:, 0, :], 0, QW, qc == 0, False)
                z_bf(ktA + 1, ptA[:, 1, 128:QW], 128, QW, False, False)
                z_bf(ktB, ptB[:, 0, 0:256], 256, QW, False, False)
                z_bf(ktB + 1, ptB[:, 0, 384:QW], 384, QW, False, True)

                recip = prs.tile([1, QW], F32R, tag="recip", name=f"rc{qc}_{h}")
                with nc.allow_low_precision(reason="softmax recip in fp32r"):
                    nc.vector.reciprocal(recip[:], zps[DH:DH + 1, :])
                # broadcast 1/denom across 64 partitions with a rank-1 matmul
                # into the free rows 64:128 of the same psum bank
                nc.tensor.matmul(
                    zps[DH:2 * DH, :], ones64[:], recip[:],
                    start=True, stop=True, skip_group_check=True,
                )
                if qc not in ZTS:
                    ZTS[qc] = [
                        pzt.tile([128, QW], F32R, tag="zt", name=f"zt{qc}_{p}")
                        for p in range(2)
                    ]
                nc.vector.tensor_mul(
                    ZTS[qc][hp][hs, :], zps[0:DH, :], zps[DH:2 * DH, :])

            # ---- global pipelined emission ----
            # proj(0): Q singles on psZ, K pair + V pairs on psS
            emit_qk_singles0()
            emit_qk_pair(0, "k")
            emit_v_pair(0)
            emit_v_pair(1)
            # head stream with PE fillers: proj(qc+1) during attention(qc),
            # O-proj(qc-1) during attention(qc+1)
            emit_head(0, 0, [lambda: emit_qk_pair(1, "q")])
            emit_head(0, 1, [lambda: emit_qk_pair(1, "k")])
            emit_head(0, 2, [lambda: emit_v_pair(2)])
            emit_head(0, 3, [lambda: emit_v_pair(3)])
            emit_head(1, 0, [lambda: emit_qk_pair(2, "q")])
            emit_head(1, 1, [lambda: emit_qk_pair(2, "k"),
                             lambda: emit_o_unit(0, 0)])
            emit_head(1, 2, [lambda: emit_v_pair(4),
                             lambda: emit_o_unit(0, 1)])
            emit_head(1, 3, [lambda: emit_v_pair(5),
                             lambda: emit_o_unit(0, 2)])
            emit_head(2, 0, [lambda: emit_qk_pair(3, "q"),
                             lambda: emit_o_unit(0, 3)])
            emit_head(2, 1, [lambda: emit_qk_pair(3, "k"),
                             lambda: emit_o_unit(1, 0)])
            emit_head(2, 2, [lambda: emit_v_pair(6),
                             lambda: emit_o_unit(1, 1)])
            emit_head(2, 3, [lambda: emit_v_pair(7),
                             lambda: emit_o_unit(1, 2)])
            emit_head(3, 0, [lambda: emit_o_unit(1, 3)])
            emit_head(3, 1, [lambda: emit_o_unit(2, 0),
                             lambda: emit_o_unit(2, 1)])
            emit_head(3, 2, [lambda: emit_o_unit(2, 2)])
            emit_head(3, 3, [lambda: emit_o_unit(2, 3)])
            for u in range(4):
                emit_o_unit(3, u)

    nc.compile()
    return nc


def _get_program(with_bias: bool):
    if with_bias not in _PROGRAMS:
        _PROGRAMS[with_bias] = _build(with_bias)
    return _PROGRAMS[with_bias]


def _split8(a):
    hi = a.astype(ml_dtypes.float8_e4m3)
    lo = (a - hi.astype(np.float32)).astype(ml_dtypes.float8_e4m3)
    return hi, lo


def _x_layout(a):
    # [1024, 2048] -> [128, j, i, qb, s]
    return np.ascontiguousarray(
        a.reshape(KCP, 2, 128, NQC, QW).transpose(2, 0, 1, 3, 4))


def _w_layout(a):
    # [1024, 256] -> [128, j, i, 256]
    return np.ascontiguousarray(
        a.reshape(KCP, 2, 128, 256).transpose(2, 0, 1, 3))


def kernel(normalized_resid_pre, W_Q, W_K, W_V, W_O, b_Q, b_K, b_V, b_O):
    x = np.asarray(normalized_resid_pre, dtype=np.float32)
    W_Q = np.asarray(W_Q, dtype=np.float32)
    W_K = np.asarray(W_K, dtype=np.float32)
    W_V = np.asarray(W_V, dtype=np.float32)
    W_O = np.asarray(W_O, dtype=np.float32)
    b_Q = np.asarray(b_Q, dtype=np.float32)
    b_K = np.asarray(b_K, dtype=np.float32)
    b_V = np.asarray(b_V, dtype=np.float32)
    b_O = np.asarray(b_O, dtype=np.float32)

    batch, seq, dm = x.shape
    with_bias = bool(np.any(b_Q) or np.any(b_K) or np.any(b_V))
    nc = _get_program(with_bias)

    tri = np.ascontiguousarray(
        np.triu(np.ones((128, 128), np.float32)).astype(ml_dtypes.bfloat16))

    xsp = []
    for b in range(batch):
        xh, xl = _split8(np.ascontiguousarray(x[b].T))
        xsp.append((_x_layout(xh), _x_layout(xl)))

    in_maps = []
    for c in range(8):
        b, g = c // 4, c % 4
        hs = slice(4 * g, 4 * g + 4)
        m = {"xh8": xsp[b][0], "xl8": xsp[b][1], "tri": tri}
        for nm, W in (("wq", W_Q), ("wk", W_K), ("wv", W_V)):
            Wp = np.transpose(W[hs], (1, 0, 2)).reshape(dm, 256) * WS
            hi, lo = _split8(Wp)
            m[nm + "h"] = _w_layout(hi)
            m[nm + "l"] = _w_layout(lo)
        m["wo"] = np.ascontiguousarray(
            (W_O[hs].reshape(256, dm) / WS).astype(ml_dtypes.bfloat16))
        if with_bias:
            m["bqkv"] = np.ascontiguousarray(np.concatenate(
                [b_Q[hs].reshape(256) * WS, b_K[hs].reshape(256) * WS,
                 b_V[hs].reshape(256) * WS]
            )[None, :].astype(ml_dtypes.bfloat16))
            m["ones"] = np.ones((1, seq), ml_dtypes.bfloat16)
        in_maps.append(m)

    res = bass_utils.run_bass_kernel_spmd(nc, in_maps, core_ids=list(range(8)))
    parts = [res.results[c]["out"] for c in range(8)]
    full = np.stack(
        [parts[0] + parts[1] + parts[2] + parts[3],
         parts[4] + parts[5] + parts[6] + parts[7]]
    )
    full += b_O
    return full.astype(np.float32)


# revision 14
# speedup vs baseline: 1.1606x; 1.0025x over previous
"""Multi-head causal attention (B=2, S=2048, D=1024, H=16, Dh=64) on 8 TRN2 cores.

Sharding: tensor-parallel over 4 head-groups x data-parallel over 2 batches.
Core c handles batch c//4, heads [4*(c%4), 4*(c%4)+4). Each core computes its
partial output projection; the host sums the 4 partials per batch (the
"all-reduce") and adds b_O.

Mixed-precision device program (fp32 PSUM accumulation everywhere):
  - Q/K/V projections: fp8e4m3 DoubleRow matmuls (0.5 cyc/row, 256-deep
    contraction per instr). Host pre-splits x and 64*W into hi/lo fp8 pairs;
    3 compensation terms xh@Wh + xl@Wh + xh@Wl recover ~bf16 accuracy.
  - Scores S = (64q)(64k): bf16 QT/KT, per-128-column causal crops. The /8
    softmax scale and the 1/4096 fp8 weight scaling fold into the ACT exp
    (scale=1/32768).
  - P = exp(S): ACT writes fp8 tiles for off-diagonal k-pairs, bf16 for the
    4 diagonal k-tiles (where P concentrates and fp8 noise would not average
    out). Triangular masks multiply bf16 diag tiles only (DVE 2x on bf16).
  - Z = P @ [64V|1]: off-diag via fp8 DoubleRow over k-tile pairs (0.25
    cyc/row/k-tile), diag via bf16-moving matmuls; row 64 accumulates the
    softmax denominator.
  - out += (Z/denom)^T @ (Wo/64): f32r stationary ZT, bf16 moving WO.
"""

import numpy as np
import ml_dtypes

import concourse.mybir as mybir
import concourse.tile as tile
from concourse import bacc
from concourse import bass_utils

F32 = mybir.dt.float32
F32R = mybir.dt.float32r
BF16 = mybir.dt.bfloat16
F8 = mybir.dt.float8e4

SEQ = 2048
DM = 1024
DH = 64
HLOC = 4          # heads per core
KCP = 4           # dmodel pair-chunks of 256
NQC = 4           # q chunks of 512
QW = 512
WS = 64.0         # fp8 weight scale
EXP_SCALE = 1.0 / (WS * WS * 8.0)
DR = mybir.MatmulPerfMode.DoubleRow

_PROGRAMS = {}


def _build(with_bias: bool):
    nc = bacc.Bacc("TRN2", target_bir_lowering=False, debug=False, num_devices=8)

    # [128, j(4), i(2), qb(4), 512] with element [p,j,i,qb,s] = xT[256j+128i+p,
    # 512qb+s]; hi/lo fp8 split of xT
    xh8 = nc.dram_tensor("xh8", [128, KCP, 2, NQC, QW], F8, kind="ExternalInput").ap()
    xl8 = nc.dram_tensor("xl8", [128, KCP, 2, NQC, QW], F8, kind="ExternalInput").ap()
    # [128, j(4), i(2), 256]: hi/lo of 64*W[256j+128i+p, m]
    wname = lambda n: nc.dram_tensor(n, [128, KCP, 2, 256], F8, kind="ExternalInput").ap()
    wqh, wql = wname("wqh"), wname("wql")
    wkh, wkl = wname("wkh"), wname("wkl")
    wvh, wvl = wname("wvh"), wname("wvl")
    wo = nc.dram_tensor("wo", [256, DM], BF16, kind="ExternalInput").ap()
    tri = nc.dram_tensor("tri", [128, 128], BF16, kind="ExternalInput").ap()
    if with_bias:
        bqkv = nc.dram_tensor("bqkv", [1, 768], BF16, kind="ExternalInput").ap()
        ones = nc.dram_tensor("ones", [1, SEQ], BF16, kind="ExternalInput").ap()
    out = nc.dram_tensor("out", [SEQ, DM], F32, kind="ExternalOutput").ap()

    with tile.TileContext(nc) as tc:
        with (
            tc.tile_pool(name="px", bufs=1) as px,
            tc.tile_pool(name="pw", bufs=1) as pw,
            tc.tile_pool(name="pqk", bufs=1) as pqk,
            tc.tile_pool(name="pv", bufs=1) as pv,
            tc.tile_pool(name="ppt8", bufs=4) as ppt8,
            tc.tile_pool(name="pptb", bufs=4) as pptb,
            tc.tile_pool(name="pzt", bufs=4) as pzt,
            tc.tile_pool(name="prs", bufs=3) as prs,
            tc.tile_pool(name="pout", bufs=4) as pout,
            tc.tile_pool(name="psS", bufs=3, space="PSUM") as psS,
            tc.tile_pool(name="psZ", bufs=2, space="PSUM") as psZ,
        ):
            # ---- SBUF tiles ----
            XH = px.tile([128, KCP, 2, NQC, QW], F8, tag="xh")
            XL = px.tile([128, KCP, 2, NQC, QW], F8, tag="xl")
            WQh = pw.tile([128, KCP, 2, 256], F8, tag="wqh")
            WQl = pw.tile([128, KCP, 2, 256], F8, tag="wql")
            WKh = pw.tile([128, KCP, 2, 256], F8, tag="wkh")
            WKl = pw.tile([128, KCP, 2, 256], F8, tag="wkl")
            WVh = pw.tile([128, KCP, 2, 256], F8, tag="wvh")
            WVl = pw.tile([128, KCP, 2, 256], F8, tag="wvl")

            # ---- input DMA: qb0 x-blocks + QK weights first ----
            nc.sync.dma_start(WQh[:], wqh)
            nc.sync.dma_start(WQl[:], wql)
            nc.sync.dma_start(XH[:, 0, :, 0, :], xh8[:, 0, :, 0, :])
            nc.sync.dma_start(XL[:, 0, :, 0, :], xl8[:, 0, :, 0, :])
            nc.sync.dma_start(XH[:, 1:4, :, 0, :], xh8[:, 1:4, :, 0, :])
            nc.sync.dma_start(XL[:, 1:4, :, 0, :], xl8[:, 1:4, :, 0, :])
            nc.sync.dma_start(WKh[:], wkh)
            nc.sync.dma_start(WKl[:], wkl)
            nc.sync.dma_start(WVh[:], wvh)
            nc.sync.dma_start(WVl[:], wvl)
            TRI = pw.tile([128, 128], BF16, tag="tri")
            nc.sync.dma_start(TRI[:], tri)
            nc.sync.dma_start(XH[:, :, :, 1, :], xh8[:, :, :, 1, :])
            nc.sync.dma_start(XL[:, :, :, 1, :], xl8[:, :, :, 1, :])
            WO = pw.tile([128, 2, DM], BF16, tag="wo")
            for hp in range(2):
                nc.sync.dma_start(WO[:, hp, :], wo[hp * 128:(hp + 1) * 128, :])
            for qb in range(2, NQC):
                nc.sync.dma_start(XH[:, :, :, qb, :], xh8[:, :, :, qb, :])
                nc.sync.dma_start(XL[:, :, :, qb, :], xl8[:, :, :, qb, :])
            if with_bias:
                BQKV = pw.tile([1, 768], BF16, tag="bqkv")
                nc.sync.dma_start(BQKV[:], bqkv)
                ONES = pw.tile([1, SEQ], BF16, tag="ones")
                nc.sync.dma_start(ONES[:], ones)

            # QT/KT bf16 [128 (head-pair), qc, 512] per hp; VB f32r with ones
            # col; V8 fp8 pairs (st=2m+i, only m<6 used off-diagonal)
            QT = [pqk.tile([128, NQC, QW], BF16, tag=f"qt{hp}", name=f"qt{hp}")
                  for hp in range(2)]
            KT = [pqk.tile([128, NQC, QW], BF16, tag=f"kt{hp}", name=f"kt{hp}")
                  for hp in range(2)]
            VB = pv.tile([128, 16, HLOC, DH + 1], F32R, tag="vb")
            V8 = pv.tile([128, 6, 2, HLOC, DH + 1], F8, tag="v8")
            nc.vector.memset(VB[:, :, :, DH:DH + 1], 1.0)
            nc.vector.memset(V8[:, :, :, :, DH:DH + 1], 1.0)
            ones64 = pw.tile([1, DH], F32R, tag="ones64")
            nc.vector.memset(ones64[:], 1.0)

            # ---- projections: fp8 DoubleRow 3-term chains ----
            def qk_chain(pp_slice, w_hi, w_lo, hp, qc, bias_off):
                for j in range(KCP):
                    for term, (w_sb, x_sb) in enumerate(
                        ((w_hi, XH), (w_hi, XL), (w_lo, XH))
                    ):
                        nc.tensor.matmul(
                            pp_slice,
                            w_sb[:, j, :, hp * 128:(hp + 1) * 128],
                            x_sb[:, j, :, qc, :],
                            start=(j == 0 and term == 0),
                            stop=(j == KCP - 1 and term == 2 and not with_bias),
                            perf_mode=DR,
                        )
                if with_bias:
                    nc.tensor.matmul(
                        pp_slice,
                        BQKV[0:1, bias_off + hp * 128:bias_off + (hp + 1) * 128],
                        ONES[0:1, qc * QW:(qc + 1) * QW],
                        start=False, stop=True,
                    )

            def v_chain(pp_slice, st):
                qb, s0 = st // 4, (st % 4) * 128
                for j in range(KCP):
                    for term, (x_sb, w_sb) in enumerate(
                        ((XH, WVh), (XL, WVh), (XH, WVl))
                    ):
                        nc.tensor.matmul(
                            pp_slice,
                            x_sb[:, j, :, qb, s0:s0 + 128],
                            w_sb[:, j, :, :],
                            start=(j == 0 and term == 0),
                            stop=(j == KCP - 1 and term == 2 and not with_bias),
                            perf_mode=DR,
                        )
                if with_bias:
                    nc.tensor.matmul(
                        pp_slice,
                        ONES[0:1, st * 128:(st + 1) * 128],
                        BQKV[0:1, 512:768],
                        start=False, stop=True,
                    )

            # ---- projection chain-tile emitters (PE filler units) ----
            def emit_qk_single0(hp):
                # qc0 Q chain as a psZ single (psZ idle until attention)
                pz = psZ.tile([128, QW], F32, tag="z", name=f"qz{hp}")
                qk_chain(pz[:], WQh, WQl, hp, 0, 0)
                nc.gpsimd.tensor_copy(QT[hp][:, 0, :], pz[:])

            def emit_qk_pair(qc, which):
                w_hi, w_lo, dst, boff = (
                    (WQh, WQl, QT, 0) if which == "q" else (WKh, WKl, KT, 256))
                pp = psS.tile([128, 2, QW], F32, tag="s", name=f"{which}p{qc}")
                for hp in range(2):
                    qk_chain(pp[:, hp, :], w_hi, w_lo, hp, qc, boff)
                for hp in range(2):
                    nc.gpsimd.tensor_copy(dst[hp][:, qc, :], pp[:, hp, :])

            def emit_v_pair(st2):
                pp = psS.tile([128, 2, QW], F32, tag="s", name=f"vp{st2}")
                for i in range(2):
                    v_chain(pp[:, i, 0:256], st2 * 2 + i)
                for i in range(2):
                    st = st2 * 2 + i
                    nc.vector.tensor_copy(
                        VB[:, st, :, 0:DH],
                        pp[:, i, 0:256].rearrange("p (h d) -> p h d", h=HLOC),
                    )
                    if st < 12:
                        nc.vector.tensor_copy(
                            V8[:, st // 2, st % 2, :, 0:DH],
                            pp[:, i, 0:256].rearrange("p (h d) -> p h d", h=HLOC),
                        )

            # ---- output projection units (PE filler; 2 (qt,mc) per tile) ----
            ZTS = {}   # qc -> [ZT_hp0, ZT_hp1]

            def emit_o_unit(qc, u):
                # u in 0..3: qt pair-of-(mc) unit -> one psS tile
                q0 = qc * QW
                qt = u
                ZT = ZTS[qc]
                t = psS.tile([128, 2, QW], F32, tag="s", name=f"o{qc}_{u}")
                osb = pout.tile([128, DM], F32, tag="ob", name=f"ob{qc}_{qt}")
                for mc in range(2):
                    for hp in range(2):
                        nc.tensor.matmul(
                            t[:, mc, :],
                            ZT[hp][:, qt * 128:(qt + 1) * 128],
                            WO[:, hp, mc * QW:(mc + 1) * QW],
                            start=(hp == 0), stop=(hp == 1),
                        )
                for mc in range(2):
                    eng = nc.vector if (qt + mc) % 2 == 0 else nc.gpsimd
                    eng.tensor_copy(osb[:, mc * QW:(mc + 1) * QW], t[:, mc, :])
                nc.sync.dma_start(
                    out[q0 + qt * 128:q0 + (qt + 1) * 128, :], osb[:])

            # ---- one attention head: scores->exp->Z with the Z matmuls
            # delayed one s-tile behind (exp latency hiding) and a PE filler
            # emitted before the diagonal Z drain ----
            def emit_head(qc, h, fillers):
                hp, hh = h // 2, h % 2
                hs = slice(hh * DH, (hh + 1) * DH)
                zps = psZ.tile([128, QW], F32, tag="z", name=f"z{qc}_{h}")

                def s_mm(dst, kt, q_lo, q_hi):
                    nc.tensor.matmul(
                        dst,
                        KT[hp][hs, kt // 4, (kt % 4) * 128:(kt % 4 + 1) * 128],
                        QT[hp][hs, qc, q_lo:q_hi],
                        start=True, stop=True,
                    )

                def z_dr(m, pt8):
                    nc.tensor.matmul(
                        zps[0:DH + 1, :], V8[:, m, :, h, :], pt8[:],
                        start=(m == 0), stop=False,
                        perf_mode=DR, skip_group_check=True,
                    )

                pend = []
                for m in range(2 * qc):
                    sps = psS.tile([128, 2, QW], F32, tag="s",
                                   name=f"s{qc}_{h}_{m}")
                    for i in range(2):
                        s_mm(sps[:, i, :], 2 * m + i, 0, QW)
                    pt8 = ppt8.tile([128, 2, QW], F8, tag="p8",
                                    name=f"p8_{qc}_{h}_{m}")
                    nc.scalar.activation(
                        pt8[:], sps[:], mybir.ActivationFunctionType.Exp,
                        scale=EXP_SCALE,
                    )
                    pend.append((z_dr, m, pt8))
                    if len(pend) > 2:
                        fn, mm, pt = pend.pop(0)
                        fn(mm, pt)

                ktA, ktB = 4 * qc, 4 * qc + 2
                sA = psS.tile([128, 2, QW], F32, tag="s", name=f"sA{qc}_{h}")
                s_mm(sA[:, 0, :], ktA, 0, QW)
                s_mm(sA[:, 1, :], ktA + 1, 0, QW)
                ptA = pptb.tile([128, 2, QW], BF16, tag="pb", name=f"pA{qc}_{h}")
                nc.scalar.activation(
                    ptA[:], sA[:], mybir.ActivationFunctionType.Exp,
                    scale=EXP_SCALE,
                )
                for fn, mm, pt in pend:
                    fn(mm, pt)
                sB = psS.tile([128, 2, QW], F32, tag="s", name=f"sB{qc}_{h}")
                s_mm(sB[:, 0, 0:256], ktB, 256, QW)
                s_mm(sB[:, 0, 256:QW], ktB + 1, 256, QW)
                ptB = pptb.tile([128, 2, QW], BF16, tag="pb", name=f"pB{qc}_{h}")
                nc.scalar.activation(
                    ptB[:, 0, :], sB[:, 0, :],
                    mybir.ActivationFunctionType.Exp, scale=EXP_SCALE,
                )
                nc.vector.tensor_mul(ptA[:, 0, 0:128], ptA[:, 0, 0:128], TRI[:])
                nc.vector.tensor_mul(ptA[:, 1, 128:256], ptA[:, 1, 128:256], TRI[:])
                nc.vector.tensor_mul(ptB[:, 0, 0:128], ptB[:, 0, 0:128], TRI[:])
                nc.vector.tensor_mul(ptB[:, 0, 384:QW], ptB[:, 0, 384:QW], TRI[:])

                # PE filler while the diagonal exps+masks complete
                for f in fillers:
                    f()

                def z_bf(v_st, pt_ap, q_lo, q_hi, start, stop):
                    nc.tensor.matmul(
                        zps[0:DH + 1, q_lo:q_hi],
                        VB[:, v_st, h, :], pt_ap,
                        start=start, stop=stop, skip_group_check=True,
                    )

                z_bf(ktA, ptA[:, 0, :], 0, QW, qc == 0, False)
                z_bf(ktA + 1, ptA[:, 1, 128:QW], 128, QW, False, False)
                z_bf(ktB, ptB[:, 0, 0:256], 256, QW, False, False)
                z_bf(ktB + 1, ptB[:, 0, 384:QW], 384, QW, False, True)

                recip = prs.tile([1, QW], F32R, tag="recip", name=f"rc{qc}_{h}")
                with nc.allow_low_precision(reason="softmax recip in fp32r"):
                    nc.vector.reciprocal(recip[:], zps[DH:DH + 1, :])
                # broadcast 1/denom across 64 partitions with a rank-1 matmul
                # into the free rows 64:128 of the same psum bank
                nc.tensor.matmul(
                    zps[DH:2 * DH, :], ones64[:], recip[:],
                    start=True, stop=True, skip_group_check=True,
                )
                if qc not in ZTS:
                    ZTS[qc] = [
                        pzt.tile([128, QW], F32R, tag="zt", name=f"zt{qc}_{p}")
                        for p in range(2)
                    ]
                nc.vector.tensor_mul(
                    ZTS[qc][hp][hs, :], zps[0:DH, :], zps[DH:2 * DH, :])

            # ---- global pipelined emission ----
            # minimal prefix so the first exp fires early: Q-hp0, K pair,
            # V pairs for qc0 diag; remaining chains ride as head fillers
            emit_qk_single0(0)
            emit_qk_pair(0, "k")
            emit_v_pair(0)
            emit_v_pair(1)
            # head stream with PE fillers: proj(qc+1) during attention(qc),
            # O-proj(qc-1) during attention(qc+1)
            emit_head(0, 0, [lambda: emit_qk_single0(1)])
            emit_head(0, 1, [lambda: emit_qk_pair(1, "q")])
            emit_head(0, 2, [lambda: emit_qk_pair(1, "k")])
            emit_head(0, 3, [lambda: emit_v_pair(2)])
            emit_head(1, 0, [lambda: emit_v_pair(3)])
            emit_head(1, 1, [lambda: emit_qk_pair(2, "q"),
                             lambda: emit_o_unit(0, 0)])
            emit_head(1, 2, [lambda: emit_qk_pair(2, "k"),
                             lambda: emit_o_unit(0, 1)])
            emit_head(1, 3, [lambda: emit_v_pair(4),
                             lambda: emit_o_unit(0, 2)])
            emit_head(2, 0, [lambda: emit_v_pair(5),
                             lambda: emit_o_unit(0, 3)])
            emit_head(2, 1, [lambda: emit_qk_pair(3, "q"),
                             lambda: emit_o_unit(1, 0)])
            emit_head(2, 2, [lambda: emit_qk_pair(3, "k"),
                             lambda: emit_o_unit(1, 1)])
            emit_head(2, 3, [lambda: emit_v_pair(6),
                             lambda: emit_o_unit(1, 2)])
            emit_head(3, 0, [lambda: emit_v_pair(7),
                             lambda: emit_o_unit(1, 3)])
            emit_head(3, 1, [lambda: emit_o_unit(2, 0),
                             lambda: emit_o_unit(2, 1)])
            emit_head(3, 2, [lambda: emit_o_unit(2, 2)])
            emit_head(3, 3, [lambda: emit_o_unit(2, 3)])
            for u in range(4):
                emit_o_unit(3, u)

    nc.compile()
    return nc


def _get_program(with_bias: bool):
    if with_bias not in _PROGRAMS:
        _PROGRAMS[with_bias] = _build(with_bias)
    return _PROGRAMS[with_bias]


def _split8(a):
    hi = a.astype(ml_dtypes.float8_e4m3)
    lo = (a - hi.astype(np.float32)).astype(ml_dtypes.float8_e4m3)
    return hi, lo


def _x_layout(a):
    # [1024, 2048] -> [128, j, i, qb, s]
    return np.ascontiguousarray(
        a.reshape(KCP, 2, 128, NQC, QW).transpose(2, 0, 1, 3, 4))


def _w_layout(a):
    # [1024, 256] -> [128, j, i, 256]
    return np.ascontiguousarray(
        a.reshape(KCP, 2, 128, 256).transpose(2, 0, 1, 3))


def kernel(normalized_resid_pre, W_Q, W_K, W_V, W_O, b_Q, b_K, b_V, b_O):
    x = np.asarray(normalized_resid_pre, dtype=np.float32)
    W_Q = np.asarray(W_Q, dtype=np.float32)
    W_K = np.asarray(W_K, dtype=np.float32)
    W_V = np.asarray(W_V, dtype=np.float32)
    W_O = np.asarray(W_O, dtype=np.float32)
    b_Q = np.asarray(b_Q, dtype=np.float32)
    b_K = np.asarray(b_K, dtype=np.float32)
    b_V = np.asarray(b_V, dtype=np.float32)
    b_O = np.asarray(b_O, dtype=np.float32)

    batch, seq, dm = x.shape
    with_bias = bool(np.any(b_Q) or np.any(b_K) or np.any(b_V))
    nc = _get_program(with_bias)

    tri = np.ascontiguousarray(
        np.triu(np.ones((128, 128), np.float32)).astype(ml_dtypes.bfloat16))

    xsp = []
    for b in range(batch):
        xh, xl = _split8(np.ascontiguousarray(x[b].T))
        xsp.append((_x_layout(xh), _x_layout(xl)))

    in_maps = []
    for c in range(8):
        b, g = c // 4, c % 4
        hs = slice(4 * g, 4 * g + 4)
        m = {"xh8": xsp[b][0], "xl8": xsp[b][1], "tri": tri}
        for nm, W in (("wq", W_Q), ("wk", W_K), ("wv", W_V)):
            Wp = np.transpose(W[hs], (1, 0, 2)).reshape(dm, 256) * WS
            hi, lo = _split8(Wp)
            m[nm + "h"] = _w_layout(hi)
            m[nm + "l"] = _w_layout(lo)
        m["wo"] = np.ascontiguousarray(
            (W_O[hs].reshape(256, dm) / WS).astype(ml_dtypes.bfloat16))
        if with_bias:
            m["bqkv"] = np.ascontiguousarray(np.concatenate(
                [b_Q[hs].reshape(256) * WS, b_K[hs].reshape(256) * WS,
                 b_V[hs].reshape(256) * WS]
            )[None, :].astype(ml_dtypes.bfloat16))
            m["ones"] = np.ones((1, seq), ml_dtypes.bfloat16)
        in_maps.append(m)

    res = bass_utils.run_bass_kernel_spmd(nc, in_maps, core_ids=list(range(8)))
    parts = [res.results[c]["out"] for c in range(8)]
    full = np.stack(
        [parts[0] + parts[1] + parts[2] + parts[3],
         parts[4] + parts[5] + parts[6] + parts[7]]
    )
    full += b_O
    return full.astype(np.float32)


# revision 25
# speedup vs baseline: 1.2909x; 1.1123x over previous
"""Multi-head causal attention (B=2, S=2048, D=1024, H=16, Dh=64) on 8 TRN2 cores.

Sharding: tensor-parallel over 4 head-groups x data-parallel over 2 batches.
Core c handles batch c//4, heads [4*(c%4), 4*(c%4)+4). Each core computes its
partial output projection; the host sums the 4 partials per batch (the
"all-reduce") and adds b_O.

Mixed-precision device program (fp32 PSUM accumulation everywhere):
  - Q/K/V projections: fp8e4m3 DoubleRow matmuls (0.5 cyc/row, 256-deep
    contraction per instr). Host pre-splits x and 64*W into hi/lo fp8 pairs;
    3 compensation terms xh@Wh + xl@Wh + xh@Wl recover ~bf16 accuracy.
  - Scores S = (64q)(64k): bf16 QT/KT, per-128-column causal crops. The /8
    softmax scale and the 1/4096 fp8 weight scaling fold into the ACT exp
    (scale=1/32768).
  - P = exp(S): ACT writes fp8 tiles for off-diagonal k-pairs, bf16 for the
    4 diagonal k-tiles (where P concentrates and fp8 noise would not average
    out). Triangular masks multiply bf16 diag tiles only (DVE 2x on bf16).
  - Z = P @ [64V|1]: off-diag via fp8 DoubleRow over k-tile pairs (0.25
    cyc/row/k-tile), diag via bf16-moving matmuls; row 64 accumulates the
    softmax denominator.
  - out += (Z/denom)^T @ (Wo/64): f32r stationary ZT, bf16 moving WO.
"""

import numpy as np
import ml_dtypes

import concourse.mybir as mybir
import concourse.tile as tile
from concourse import bacc
from concourse import bass_utils

F32 = mybir.dt.float32
F32R = mybir.dt.float32r
BF16 = mybir.dt.bfloat16
F8 = mybir.dt.float8e4

SEQ = 2048
DM = 1024
DH = 64
HLOC = 4          # heads per core
KCP = 4           # dmodel pair-chunks of 256
NQC = 4           # q chunks of 512
QW = 512
WS = 64.0         # fp8 weight scale
EXP_SCALE = 1.0 / (WS * WS * 8.0)
DR = mybir.MatmulPerfMode.DoubleRow

_PROGRAMS = {}


def _build(with_bias: bool):
    nc = bacc.Bacc("TRN2", target_bir_lowering=False, debug=False, num_devices=8)

    # [128, j(4), i(2), qb(4), 512] with element [p,j,i,qb,s] = xT[256j+128i+p,
    # 512qb+s]; hi/lo fp8 split of xT
    xh8 = nc.dram_tensor("xh8", [128, KCP, 2, NQC, QW], F8, kind="ExternalInput").ap()
    xl8 = nc.dram_tensor("xl8", [128, KCP, 2, NQC, QW], F8, kind="ExternalInput").ap()
    # [128, j(4), i(2), 256]: hi/lo of 64*W[256j+128i+p, m]
    wname = lambda n: nc.dram_tensor(n, [128, KCP, 2, 256], F8, kind="ExternalInput").ap()
    wqh, wql = wname("wqh"), wname("wql")
    wkh, wkl = wname("wkh"), wname("wkl")
    wvh, wvl = wname("wvh"), wname("wvl")
    wo = nc.dram_tensor("wo", [256, DM], BF16, kind="ExternalInput").ap()
    tri = nc.dram_tensor("tri", [128, 128], BF16, kind="ExternalInput").ap()
    if with_bias:
        bqkv = nc.dram_tensor("bqkv", [1, 768], BF16, kind="ExternalInput").ap()
        ones = nc.dram_tensor("ones", [1, SEQ], BF16, kind="ExternalInput").ap()
    out = nc.dram_tensor("out", [SEQ, DM], F32, kind="ExternalOutput").ap()

    with tile.TileContext(nc) as tc:
        with (
            tc.tile_pool(name="px", bufs=1) as px,
            tc.tile_pool(name="pw", bufs=1) as pw,
            tc.tile_pool(name="pqk", bufs=1) as pqk,
            tc.tile_pool(name="pv", bufs=1) as pv,
            tc.tile_pool(name="ppt8", bufs=4) as ppt8,
            tc.tile_pool(name="pptb", bufs=4) as pptb,
            tc.tile_pool(name="pzt", bufs=4) as pzt,
            tc.tile_pool(name="prs", bufs=3) as prs,
            tc.tile_pool(name="pout", bufs=4) as pout,
            tc.tile_pool(name="psS", bufs=2, space="PSUM") as psS,
            tc.tile_pool(name="psZ", bufs=2, space="PSUM") as psZ,
            tc.tile_pool(name="psF", bufs=2, space="PSUM") as psF,
        ):
            # ---- SBUF tiles ----
            XH = px.tile([128, KCP, 2, NQC, QW], F8, tag="xh")
            XL = px.tile([128, KCP, 2, NQC, QW], F8, tag="xl")
            WQh = pw.tile([128, KCP, 2, 256], F8, tag="wqh")
            WQl = pw.tile([128, KCP, 2, 256], F8, tag="wql")
            WKh = pw.tile([128, KCP, 2, 256], F8, tag="wkh")
            WKl = pw.tile([128, KCP, 2, 256], F8, tag="wkl")
            WVh = pw.tile([128, KCP, 2, 256], F8, tag="wvh")
            WVl = pw.tile([128, KCP, 2, 256], F8, tag="wvl")

            # ---- input DMA: qb0 x-blocks + QK weights first ----
            nc.sync.dma_start(WQh[:], wqh)
            nc.sync.dma_start(WQl[:], wql)
            nc.sync.dma_start(XH[:, 0, :, 0, :], xh8[:, 0, :, 0, :])
            nc.sync.dma_start(XL[:, 0, :, 0, :], xl8[:, 0, :, 0, :])
            nc.sync.dma_start(XH[:, 1:4, :, 0, :], xh8[:, 1:4, :, 0, :])
            nc.sync.dma_start(XL[:, 1:4, :, 0, :], xl8[:, 1:4, :, 0, :])
            nc.sync.dma_start(WKh[:], wkh)
            nc.sync.dma_start(WKl[:], wkl)
            nc.sync.dma_start(WVh[:], wvh)
            nc.sync.dma_start(WVl[:], wvl)
            TRI = pw.tile([128, 128], BF16, tag="tri")
            nc.sync.dma_start(TRI[:], tri)
            nc.sync.dma_start(XH[:, :, :, 1, :], xh8[:, :, :, 1, :])
            nc.sync.dma_start(XL[:, :, :, 1, :], xl8[:, :, :, 1, :])
            WO = pw.tile([128, 2, DM], BF16, tag="wo")
            for hp in range(2):
                nc.sync.dma_start(WO[:, hp, :], wo[hp * 128:(hp + 1) * 128, :])
            for qb in range(2, NQC):
                nc.sync.dma_start(XH[:, :, :, qb, :], xh8[:, :, :, qb, :])
                nc.sync.dma_start(XL[:, :, :, qb, :], xl8[:, :, :, qb, :])
            if with_bias:
                BQKV = pw.tile([1, 768], BF16, tag="bqkv")
                nc.sync.dma_start(BQKV[:], bqkv)
                ONES = pw.tile([1, SEQ], BF16, tag="ones")
                nc.sync.dma_start(ONES[:], ones)

            # QT/KT bf16 [128 (head-pair), qc, 512] per hp; VB f32r with ones
            # col; V8 fp8 pairs (st=2m+i, only m<6 used off-diagonal)
            QT = [pqk.tile([128, NQC, QW], BF16, tag=f"qt{hp}", name=f"qt{hp}")
                  for hp in range(2)]
            KT = [pqk.tile([128, NQC, QW], BF16, tag=f"kt{hp}", name=f"kt{hp}")
                  for hp in range(2)]
            VB = pv.tile([128, 16, HLOC, DH + 1], F32R, tag="vb")
            V8 = pv.tile([128, 6, 2, HLOC, DH + 1], F8, tag="v8")
            nc.vector.memset(VB[:, :, :, DH:DH + 1], 1.0)
            nc.vector.memset(V8[:, :, :, :, DH:DH + 1], 1.0)
            ones64 = pw.tile([1, DH], F32R, tag="ones64")
            nc.vector.memset(ones64[:], 1.0)

            # ---- projections: fp8 DoubleRow 3-term chains (generators
            # yielding after each matmul so fillers can interleave) ----
            def qk_chain_g(pp_slice, w_hi, w_lo, hp, qc, bias_off):
                for j in range(KCP):
                    for term, (w_sb, x_sb) in enumerate(
                        ((w_hi, XH), (w_hi, XL), (w_lo, XH))
                    ):
                        nc.tensor.matmul(
                            pp_slice,
                            w_sb[:, j, :, hp * 128:(hp + 1) * 128],
                            x_sb[:, j, :, qc, :],
                            start=(j == 0 and term == 0),
                            stop=(j == KCP - 1 and term == 2 and not with_bias),
                            perf_mode=DR,
                        )
                        yield
                if with_bias:
                    nc.tensor.matmul(
                        pp_slice,
                        BQKV[0:1, bias_off + hp * 128:bias_off + (hp + 1) * 128],
                        ONES[0:1, qc * QW:(qc + 1) * QW],
                        start=False, stop=True,
                    )
                    yield

            def v_chain_g(pp_slice, st):
                qb, s0 = st // 4, (st % 4) * 128
                for j in range(KCP):
                    for term, (x_sb, w_sb) in enumerate(
                        ((XH, WVh), (XL, WVh), (XH, WVl))
                    ):
                        nc.tensor.matmul(
                            pp_slice,
                            x_sb[:, j, :, qb, s0:s0 + 128],
                            w_sb[:, j, :, :],
                            start=(j == 0 and term == 0),
                            stop=(j == KCP - 1 and term == 2 and not with_bias),
                            perf_mode=DR,
                        )
                        yield
                if with_bias:
                    nc.tensor.matmul(
                        pp_slice,
                        ONES[0:1, st * 128:(st + 1) * 128],
                        BQKV[0:1, 512:768],
                        start=False, stop=True,
                    )
                    yield

            class Filler:
                """FIFO of PE micro-work generators; m-loops pull a few
                matmuls at a time so proj/O-proj work interleaves with the
                ACT-paced attention stream."""

                def __init__(self):
                    self.units = []
                    self.cur = None
                    self.cur_marker = None

                def add(self, marker, genfn):
                    self.units.append((marker, genfn))

                def pull(self, n):
                    while n > 0:
                        if self.cur is None:
                            if not self.units:
                                return
                            self.cur_marker, genfn = self.units.pop(0)
                            self.cur = genfn()
                        try:
                            next(self.cur)
                            n -= 1
                        except StopIteration:
                            self.cur = None

                def drain_marker(self, marker):
                    while (self.cur is not None and self.cur_marker == marker) \
                            or any(m == marker for m, _ in self.units):
                        self.pull(1)

            # ---- projection chain-tile emitters (PE filler units) ----
            def qk_single_g(qc, hp, which):
                # Q/K chain as a psF single, generator form
                w_hi, w_lo, dst, boff = (
                    (WQh, WQl, QT, 0) if which == "q" else (WKh, WKl, KT, 256))
                pz = psF.tile([128, QW], F32, tag="f",
                              name=f"f{which}{qc}_{hp}")
                yield from qk_chain_g(pz[:], w_hi, w_lo, hp, qc, boff)
                nc.gpsimd.tensor_copy(dst[hp][:, qc, :], pz[:])

            def v_single_g(st):
                pz = psF.tile([128, QW], F32, tag="f", name=f"fv{st}")
                yield from v_chain_g(pz[:, 0:256], st)
                nc.vector.tensor_copy(
                    VB[:, st, :, 0:DH],
                    pz[:, 0:256].rearrange("p (h d) -> p h d", h=HLOC),
                )
                if st < 12:
                    nc.vector.tensor_copy(
                        V8[:, st // 2, st % 2, :, 0:DH],
                        pz[:, 0:256].rearrange("p (h d) -> p h d", h=HLOC),
                    )

            def emit_qk_pair(qc, which):
                w_hi, w_lo, dst, boff = (
                    (WQh, WQl, QT, 0) if which == "q" else (WKh, WKl, KT, 256))
                pp = psS.tile([128, 2, QW], F32, tag="s", name=f"{which}p{qc}")
                for hp in range(2):
                    for _ in qk_chain_g(pp[:, hp, :], w_hi, w_lo, hp, qc, boff):
                        pass
                for hp in range(2):
                    nc.gpsimd.tensor_copy(dst[hp][:, qc, :], pp[:, hp, :])

            def emit_v_pair(st2):
                pp = psS.tile([128, 2, QW], F32, tag="s", name=f"vp{st2}")
                for i in range(2):
                    for _ in v_chain_g(pp[:, i, 0:256], st2 * 2 + i):
                        pass
                for i in range(2):
                    st = st2 * 2 + i
                    nc.vector.tensor_copy(
                        VB[:, st, :, 0:DH],
                        pp[:, i, 0:256].rearrange("p (h d) -> p h d", h=HLOC),
                    )
                    if st < 12:
                        nc.vector.tensor_copy(
                            V8[:, st // 2, st % 2, :, 0:DH],
                            pp[:, i, 0:256].rearrange("p (h d) -> p h d", h=HLOC),
                        )

            # ---- output projection units (PE filler; one (qt,mc) each) ----
            ZTS = {}   # qc -> [ZT_hp0, ZT_hp1]
            OSB = {}   # (qc, qt) -> staging tile; DMA fires after mc=1

            def o_single_g(qc, qt, mc):
                q0 = qc * QW
                ZT = ZTS[qc]
                t = psF.tile([128, QW], F32, tag="f", name=f"o{qc}_{qt}_{mc}")
                if mc == 0:
                    OSB[(qc, qt)] = pout.tile([128, DM], F32, tag="ob",
                                              name=f"ob{qc}_{qt}")
                osb = OSB[(qc, qt)]
                for hp in range(2):
                    nc.tensor.matmul(
                        t[:],
                        ZT[hp][:, qt * 128:(qt + 1) * 128],
                        WO[:, hp, mc * QW:(mc + 1) * QW],
                        start=(hp == 0), stop=(hp == 1),
                    )
                    yield
                eng = nc.vector if (qt + mc) % 2 == 0 else nc.gpsimd
                eng.tensor_copy(osb[:, mc * QW:(mc + 1) * QW], t[:])
                if mc == 1:
                    nc.sync.dma_start(
                        out[q0 + qt * 128:q0 + (qt + 1) * 128, :], osb[:])

            def emit_o_unit(qc, qt):
                for mc in range(2):
                    for _ in o_single_g(qc, qt, mc):
                        pass

            # ---- one attention head: scores->exp->Z with the Z matmuls
            # delayed one s-tile behind (exp latency hiding) and filler
            # matmuls pulled between iterations ----
            def emit_head(qc, h, filler):
                hp, hh = h // 2, h % 2
                hs = slice(hh * DH, (hh + 1) * DH)
                zps = psZ.tile([128, QW], F32, tag="z", name=f"z{qc}_{h}")

                def s_mm(dst, kt, q_lo, q_hi):
                    nc.tensor.matmul(
                        dst,
                        KT[hp][hs, kt // 4, (kt % 4) * 128:(kt % 4 + 1) * 128],
                        QT[hp][hs, qc, q_lo:q_hi],
                        start=True, stop=True,
                    )

                def z_dr(m, pt8):
                    nc.tensor.matmul(
                        zps[0:DH + 1, :], V8[:, m, :, h, :], pt8[:],
                        start=(m == 0), stop=False,
                        perf_mode=DR, skip_group_check=True,
                    )

                pend = []
                for m in range(2 * qc):
                    sps = psS.tile([128, 2, QW], F32, tag="s",
                                   name=f"s{qc}_{h}_{m}")
                    for i in range(2):
                        s_mm(sps[:, i, :], 2 * m + i, 0, QW)
                    pt8 = ppt8.tile([128, 2, QW], F8, tag="p8",
                                    name=f"p8_{qc}_{h}_{m}")
                    nc.scalar.activation(
                        pt8[:], sps[:], mybir.ActivationFunctionType.Exp,
                        scale=EXP_SCALE,
                    )
                    pend.append((z_dr, m, pt8))
                    filler.pull(2)
                    if len(pend) > 1:
                        fn, mm, pt = pend.pop(0)
                        fn(mm, pt)

                ktA, ktB = 4 * qc, 4 * qc + 2
                sA = psS.tile([128, 2, QW], F32, tag="s", name=f"sA{qc}_{h}")
                s_mm(sA[:, 0, :], ktA, 0, QW)
                s_mm(sA[:, 1, :], ktA + 1, 0, QW)
                ptA = pptb.tile([128, 2, QW], BF16, tag="pb", name=f"pA{qc}_{h}")
                nc.scalar.activation(
                    ptA[:], sA[:], mybir.ActivationFunctionType.Exp,
                    scale=EXP_SCALE,
                )
                for fn, mm, pt in pend:
                    fn(mm, pt)
                sB = psS.tile([128, 2, QW], F32, tag="s", name=f"sB{qc}_{h}")
                s_mm(sB[:, 0, 0:256], ktB, 256, QW)
                s_mm(sB[:, 0, 256:QW], ktB + 1, 256, QW)
                ptB = pptb.tile([128, 2, QW], BF16, tag="pb", name=f"pB{qc}_{h}")
                nc.scalar.activation(
                    ptB[:, 0, :], sB[:, 0, :],
                    mybir.ActivationFunctionType.Exp, scale=EXP_SCALE,
                )
                nc.vector.tensor_mul(ptA[:, 0, 0:128], ptA[:, 0, 0:128], TRI[:])
                nc.vector.tensor_mul(ptA[:, 1, 128:256], ptA[:, 1, 128:256], TRI[:])
                nc.vector.tensor_mul(ptB[:, 0, 0:128], ptB[:, 0, 0:128], TRI[:])
                nc.vector.tensor_mul(ptB[:, 0, 384:QW], ptB[:, 0, 384:QW], TRI[:])

                # PE filler while the diagonal exps+masks complete
                filler.pull(8 if qc == 0 else 4)

                def z_bf(v_st, pt_ap, q_lo, q_hi, start, stop):
                    nc.tensor.matmul(
                        zps[0:DH + 1, q_lo:q_hi],
                        VB[:, v_st, h, :], pt_ap,
                        start=start, stop=stop, skip_group_check=True,
                    )

                z_bf(ktA, ptA[:, 0, :], 0, QW, qc == 0, False)
                z_bf(ktA + 1, ptA[:, 1, 128:QW], 128, QW, False, False)
                z_bf(ktB, ptB[:, 0, 0:256], 256, QW, False, False)
                z_bf(ktB + 1, ptB[:, 0, 384:QW], 384, QW, False, True)

                recip = prs.tile([1, QW], F32R, tag="recip", name=f"rc{qc}_{h}")
                with nc.allow_low_precision(reason="softmax recip in fp32r"):
                    nc.vector.reciprocal(recip[:], zps[DH:DH + 1, :])
                # broadcast 1/denom across 64 partitions with a rank-1 matmul
                # into the free rows 64:128 of the same psum bank
                nc.tensor.matmul(
                    zps[DH:2 * DH, :], ones64[:], recip[:],
                    start=True, stop=True, skip_group_check=True,
                )
                if qc not in ZTS:
                    ZTS[qc] = [
                        pzt.tile([128, QW], F32R, tag="zt", name=f"zt{qc}_{p}")
                        for p in range(2)
                    ]
                nc.vector.tensor_mul(
                    ZTS[qc][hp][hs, :], zps[0:DH, :], zps[DH:2 * DH, :])

            # ---- global pipelined emission ----
            # minimal prefix so the first exp fires early: Q-hp0, K pair,
            # V pairs for qc0 diag; everything else rides the filler queue
            for _ in qk_single_g(0, 0, "q"):
                pass
            emit_qk_pair(0, "k")
            for st in range(4):
                for _ in v_single_g(st):
                    pass

            filler = Filler()
            filler.add("p0", lambda: qk_single_g(0, 1, "q"))
            for qcn in (1, 2, 3):
                for which in ("q", "k"):
                    for hp in range(2):
                        filler.add(f"p{qcn}",
                                   (lambda qcn=qcn, hp=hp, which=which:
                                    qk_single_g(qcn, hp, which)))
                for st in range(4 * qcn, 4 * qcn + 4):
                    filler.add(f"p{qcn}", lambda st=st: v_single_g(st))
                for qt in range(4):
                    for mc in range(2):
                        filler.add(f"o{qcn-1}",
                                   (lambda qcp=qcn - 1, qt=qt, mc=mc:
                                    o_single_g(qcp, qt, mc)))

            for qc in range(NQC):
                filler.drain_marker(f"p{qc}")
                for h in range(HLOC):
                    if qc == 0 and h == 2:
                        filler.drain_marker("p0")  # QT hp1 needed
                    emit_head(qc, h, filler)
            filler.drain_marker("o2")
            for qt in range(4):
                emit_o_unit(3, qt)

    nc.compile()
    return nc


def _get_program(with_bias: bool):
    if with_bias not in _PROGRAMS:
        _PROGRAMS[with_bias] = _build(with_bias)
    return _PROGRAMS[with_bias]


def _split8(a):
    hi = a.astype(ml_dtypes.float8_e4m3)
    lo = (a - hi.astype(np.float32)).astype(ml_dtypes.float8_e4m3)
    return hi, lo


def _x_layout(a):
    # [1024, 2048] -> [128, j, i, qb, s]
    return np.ascontiguousarray(
        a.reshape(KCP, 2, 128, NQC, QW).transpose(2, 0, 1, 3, 4))


def _w_layout(a):
    # [1024, 256] -> [128, j, i, 256]
    return np.ascontiguousarray(
        a.reshape(KCP, 2, 128, 256).transpose(2, 0, 1, 3))


def kernel(normalized_resid_pre, W_Q, W_K, W_V, W_O, b_Q, b_K, b_V, b_O):
    x = np.asarray(normalized_resid_pre, dtype=np.float32)
    W_Q = np.asarray(W_Q, dtype=np.float32)
    W_K = np.asarray(W_K, dtype=np.float32)
    W_V = np.asarray(W_V, dtype=np.float32)
    W_O = np.asarray(W_O, dtype=np.float32)
    b_Q = np.asarray(b_Q, dtype=np.float32)
    b_K = np.asarray(b_K, dtype=np.float32)
    b_V = np.asarray(b_V, dtype=np.float32)
    b_O = np.asarray(b_O, dtype=np.float32)

    batch, seq, dm = x.shape
    with_bias = bool(np.any(b_Q) or np.any(b_K) or np.any(b_V))
    nc = _get_program(with_bias)

    tri = np.ascontiguousarray(
        np.triu(np.ones((128, 128), np.float32)).astype(ml_dtypes.bfloat16))

    xsp = []
    for b in range(batch):
        xh, xl = _split8(np.ascontiguousarray(x[b].T))
        xsp.append((_x_layout(xh), _x_layout(xl)))

    in_maps = []
    for c in range(8):
        b, g = c // 4, c % 4
        hs = slice(4 * g, 4 * g + 4)
        m = {"xh8": xsp[b][0], "xl8": xsp[b][1], "tri": tri}
        for nm, W in (("wq", W_Q), ("wk", W_K), ("wv", W_V)):
            Wp = np.transpose(W[hs], (1, 0, 2)).reshape(dm, 256) * WS
            hi, lo = _split8(Wp)
            m[nm + "h"] = _w_layout(hi)
            m[nm + "l"] = _w_layout(lo)
        m["wo"] = np.ascontiguousarray(
            (W_O[hs].reshape(256, dm) / WS).astype(ml_dtypes.bfloat16))
        if with_bias:
            m["bqkv"] = np.ascontiguousarray(np.concatenate(
                [b_Q[hs].reshape(256) * WS, b_K[hs].reshape(256) * WS,
                 b_V[hs].reshape(256) * WS]
            )[None, :].astype(ml_dtypes.bfloat16))
            m["ones"] = np.ones((1, seq), ml_dtypes.bfloat16)
        in_maps.append(m)

    res = bass_utils.run_bass_kernel_spmd(nc, in_maps, core_ids=list(range(8)))
    parts = [res.results[c]["out"] for c in range(8)]
    full = np.stack(
        [parts[0] + parts[1] + parts[2] + parts[3],
         parts[4] + parts[5] + parts[6] + parts[7]]
    )
    full += b_O
    return full.astype(np.float32)
